# revision 1
# baseline (speedup 1.0000x reference)
"""Trainium2 Bass kernel for nn_Generator_34127810134219 (gnn_message_passing).

Strategy
--------
The reference relmod builds a [B,N,N] score matrix S = c*x@x^T (diag masked)
and computes wr*(S@U)/N + x.  Algebraically (verified to 4e-7 rel err):

    S@U = c*( x @ (x^T U) - ||x_i||^2 * U_i )

which collapses O(B*N^2*D) work into O(B*N*D^2).  The whole network is then a
memory-light pointwise/matmul pipeline over B*N = 32768 tokens with feature
dims <= 12.

Sharding: data-parallel over batch, 2 batches per core (8 cores).  The only
cross-core coupling is BatchNorm statistics (mean/var per n over batch and
feature dims) - exchanged as tiny [8,512] partial-sum tiles via AllGather
(3x), then reduced locally.  relmod is fully batch-local.

On-chip layout: feature-major, group-packed.  Per core 4096 tokens are split
into 8 groups of 512; group g lives on partitions [16g, 16g+C).  All fc
layers become single 128x512 matmuls with block-diagonal weights (float32r
for full-rate PE).  The per-batch Gram matrix G = x^T U is built with PE
transposes + matmuls; per-group partials are folded per batch as
mask . (Phi^T P_masked Phi) . mask with a fold matrix Phi - PE matmuls only,
no cross-partition vector ops.

All constant patterns (selectors, masks, Phi, block-diagonal weights) are
built on-chip from affine_select primitives + PE matmuls: DMA triggers are
the dominant fixed cost (~0.6us each on the shared HWDGE) so the kernel
issues only ~40 of them.
"""

import numpy as np

import concourse.bass as bass
import concourse.bacc as bacc
import concourse.tile as tile
import concourse.mybir as mybir
from concourse.bass_utils import run_bass_kernel_spmd
from concourse.masks import make_identity

FP32 = mybir.dt.float32
F32R = mybir.dt.float32r
AF = mybir.ActivationFunctionType
OP = mybir.AluOpType

B, N, F = 16, 2048, 3
D2, D4 = 6, 12
NCORES = 8
BPC = B // NCORES          # batches per core
T = BPC * N                # tokens per core
NG = 8                     # groups per core
L = T // NG                # free-dim length (512)
GS = 16                    # partition stride per group
EPS = 1e-5
SAFE_RSTD = False     # Ln+Exp instead of Abs_reciprocal_sqrt
SAFE_SIGMOID = True   # ACT Sigmoid instead of exp+reciprocal_approx

# (name, shape) of every external input except x
PARAM_SPECS = [
    ("fc1_w", (D2, F)), ("fc1_b", (D2,)), ("bn1_g", (N,)), ("bn1_b", (N,)),
    ("fc2_w", (D4, D2)), ("fc2_b", (D4,)), ("bn2_g", (N,)), ("bn2_b", (N,)),
    ("fc3_w", (D4, D4)), ("fc3_b", (D4,)),
    ("u1_w", (D4, D4)), ("u1_b", (D4,)), ("ps1", (1,)), ("ph1", (1,)), ("wr1", (1,)),
    ("u2_w", (D4, D4)), ("u2_b", (D4,)), ("ps2", (1,)), ("ph2", (1,)), ("wr2", (1,)),
    ("u3_w", (D4, D4)), ("u3_b", (D4,)), ("ps3", (1,)), ("ph3", (1,)), ("wr3", (1,)),
    ("u4_w", (D4, D4)), ("u4_b", (D4,)), ("ps4", (1,)), ("ph4", (1,)), ("wr4", (1,)),
    ("fc4_w", (D2, D4)), ("fc4_b", (D2,)), ("bn4_g", (N,)), ("bn4_b", (N,)),
    ("fc5_w", (F, D2)), ("fc5_b", (F,)),
    ("fc6_w", (1, F)), ("fc6_b", (1,)), ("fc7_w", (2, F)), ("fc7_b", (2,)),
]

# weight slot order inside the compact tile (each slot is 16 cols)
W_ORDER = ["fc1_w", "fc2_w", "fc3_w", "u1_w", "u2_w", "u3_w", "u4_w",
           "fc4_w", "fc5_w", "fc67_w"]
B_ORDER = ["fc1_b", "fc2_b", "fc3_b", "u1_b", "u2_b", "u3_b", "u4_b",
           "fc4_b", "fc5_b", "fc67_b"]


def _build(single_core=False):
    nc = bacc.Bacc(
        "TRN2",
        target_bir_lowering=False,
        debug=False,
        enable_asserts=False,
        num_devices=1 if single_core else NCORES,
    )

    x_d = nc.dram_tensor("x", [BPC, N, F], FP32, kind="ExternalInput")
    prm = {
        name: nc.dram_tensor(name, list(shape), FP32, kind="ExternalInput")
        for name, shape in PARAM_SPECS
    }
    out_d = nc.dram_tensor("out", [BPC, N, F], FP32, kind="ExternalOutput")

    with tile.TileContext(nc) as tc:
        with (
            tc.tile_pool(name="consts", bufs=1) as cp,
            tc.tile_pool(name="sb", bufs=1) as sb,
            tc.tile_pool(name="pp", bufs=1, space="PSUM") as pp,
            tc.tile_pool(name="dram", bufs=1, space="DRAM") as dr,
        ):
            _emit(nc, tc, cp, sb, pp, dr, x_d, prm, out_d,
                  single_core=single_core)

    nc.compile()
    return nc


def _emit(nc, tc, cp, sb, pp, dr, x_d, prm, out_d, single_core=False):
    def mmr(out, lhsT, rhs, **kw):
        """float32r matmul: full-rate PE for fp32 bits (reduced mult precision)."""
        nc.tensor.matmul(out, lhsT.bitcast(F32R), rhs.bitcast(F32R), **kw)

    def r(ap):
        """f32r view for producer outputs feeding f32r matmuls (rounds)."""
        return ap.bitcast(F32R)

    eps_t = cp.tile([128, 1], FP32, name="eps_t")
    nc.gpsimd.memset(eps_t[:], EPS)
    # first ACT instruction uses Ln so walrus resolves the
    # natural_log_exp_and_others table set once for the whole kernel
    actwarm = sb.tile([1, 1], FP32, name="actwarm")
    nc.scalar.activation(actwarm[:], eps_t[0:1, :],
                         AF.Ln if SAFE_RSTD else AF.Abs_reciprocal_sqrt)

    # ================= affine-built base selectors (Pool engine) =============
    def affine_sel(t, pattern, cm):
        """t := 1.0 where cm*p + pattern.idx == 0 else 0."""
        nc.gpsimd.memset(t, 0.0)
        nc.gpsimd.affine_select(
            out=t, in_=t, compare_op=OP.not_equal, fill=1.0,
            base=0, pattern=pattern, channel_multiplier=cm)

    # input load first so the network isn't gated on init DMAs
    X = sb.tile([128, L], FP32, name="X")
    nc.gpsimd.memset(X[:], 0.0)
    for g in range(NG):
        b, n0 = g // 4, (g % 4) * L
        eng = nc.sync
        eng.dma_start(X[GS * g:GS * g + F, :],
                      x_d[b, n0:n0 + L, :].rearrange("n c -> c n"))

    # bc8[g, (g',c)] = [g'==g]
    bc8 = cp.tile([NG, 128], FP32, name="bc8")
    affine_sel(bc8[:].rearrange("p (g c) -> p g c", c=GS), [[1, NG], [0, GS]], -1)
    # bc4[j, (g,c)] = [g%4==j]
    bc4 = cp.tile([4, 128], FP32, name="bc4")
    affine_sel(bc4[:].rearrange("p (h j c) -> p h j c", j=4, c=GS),
               [[0, 2], [1, 4], [0, GS]], -1)
    # bcB[b, (g,c)] = [g//4==b]
    bcB = cp.tile([2, 128], FP32, name="bcB")
    affine_sel(bcB[:].rearrange("p (b j c) -> p b j c", j=4, c=GS),
               [[1, 2], [0, 4], [0, GS]], -1)
    # RepSel12[ci', (g,ci)] = [ci==ci'] (ci'<12)
    rsel12 = cp.tile([D4, 128], FP32, name="rsel12")
    affine_sel(rsel12[:].rearrange("p (g c) -> p g c", c=GS), [[0, NG], [1, GS]], -1)
    # RepSel16
    rsel16 = cp.tile([GS, 128], FP32, name="rsel16")
    affine_sel(rsel16[:].rearrange("p (g c) -> p g c", c=GS), [[0, NG], [1, GS]], -1)
    # S8[j, (r,j')] = [j'==j]  (for rank-reduction tiles)
    s8 = cp.tile([8, 64], FP32, name="s8")
    affine_sel(s8[:].rearrange("p (r j) -> p r j", j=8), [[0, 8], [1, 8]], -1)

    ident128 = cp.tile([128, 128], FP32, name="ident128")
    make_identity(nc, ident128[:])
    ones12 = cp.tile([D4, 1], FP32, name="ones12")
    nc.gpsimd.memset(ones12[:], 1.0)
    ones1 = cp.tile([1, 128], FP32, name="ones1")
    nc.gpsimd.memset(ones1[:], 1.0)
    # ================= PE-derived constant tiles =============================
    # mask_diag[(g,c),(g',c')] = [g==g']
    mask_ps = pp.tile([128, 128], FP32, name="mask_ps", tag="b0", padded_shape=[128, L])
    nc.tensor.matmul(mask_ps[:], bc8[:], bc8[:])
    mask_diag = cp.tile([128, 128], FP32, name="mask_diag")
    nc.scalar.activation(mask_diag[:], mask_ps[:], AF.Copy)
    # onesfold [128,4] = bc4^T (needed by the first bn_send pack matmuls)
    of_ps = pp.tile([128, 4], FP32, name="of_ps", tag="b3", padded_shape=[128, L])
    nc.tensor.transpose(of_ps[:], bc4[:], ident128[0:4, 0:4])
    onesfold = cp.tile([128, 4], FP32, name="onesfold")
    nc.scalar.activation(r(onesfold[:]), of_ps[:], AF.Copy)
    # deferred consts (phi/ones_c16/colmask12) are emitted in the bn1
    # AllGather window so they don't sit ahead of fc1 in the PE queue
    phi = cp.tile([128, 128], FP32, name="phi")
    ones_c16 = cp.tile([128, NG], FP32, name="ones_c16")
    colmask12 = cp.tile([128, 1], FP32, name="colmask12")

    def build_deferred_consts():
        crep_ps = pp.tile([128, 128], FP32, name="crep_ps", tag="b1",
                          padded_shape=[128, L])
        nc.tensor.matmul(crep_ps[:], rsel16[:], rsel16[:])
        crep = sb.tile([128, 128], FP32, name="crep")
        nc.scalar.activation(crep[:], crep_ps[:], AF.Copy)
        bmask_ps = pp.tile([128, 128], FP32, name="bmask_ps", tag="b2",
                           padded_shape=[128, L])
        nc.tensor.matmul(bmask_ps[:], bcB[:], bcB[:])
        nc.vector.tensor_tensor(r(phi[:]), bmask_ps[:], crep[:], OP.mult)
        oc_ps = pp.tile([128, NG], FP32, name="oc_ps", tag="b4",
                        padded_shape=[128, L])
        nc.tensor.transpose(oc_ps[:], bc8[:], ident128[0:NG, 0:NG])
        nc.scalar.activation(r(ones_c16[:]), oc_ps[:], AF.Copy)
        cm_ps = pp.tile([128, 1], FP32, name="cm_ps", tag="b5",
                        padded_shape=[128, L])
        nc.tensor.matmul(cm_ps[:], rsel12[:], ones12[:])
        nc.scalar.activation(colmask12[:], cm_ps[:], AF.Copy)
    # f32r-rounded copies of bc4/bc8 (mmr operands must have f32r producers)
    bc4r = cp.tile([4, 128], FP32, name="bc4r")
    nc.vector.tensor_copy(r(bc4r[:]), bc4[:])
    bc8r = cp.tile([NG, 128], FP32, name="bc8r")
    nc.vector.tensor_copy(r(bc8r[:]), bc8[:])
    # rank-reduction tiles [64,8] = S8^T scaled by 1/count
    rr_ps = pp.tile([64, 8], FP32, name="rr_ps", tag="b6", padded_shape=[128, L])
    nc.tensor.transpose(rr_ps[:], s8[:], ident128[0:8, 0:8])
    rr96 = cp.tile([64, 8], FP32, name="rr96")
    nc.scalar.activation(r(rr96[:]), rr_ps[:], AF.Copy, scale=1.0 / 96.0)
    rr192 = cp.tile([64, 8], FP32, name="rr192")
    nc.scalar.activation(r(rr192[:]), rr_ps[:], AF.Copy, scale=1.0 / 192.0)

    # ================= weights / biases ======================================
    # per-weight: Wc[ci,co] -DMA-> [12,16] tile; tp = Wc^T.rsel12 gives the
    # partition-replicated transpose; sp = tp^T.rsel16 spreads along free;
    # mask leaves the block-diagonal lhsT.  build_weight() is emitted at
    # chosen points so init work hides inside collective-wait windows
    # (engines run their streams in order).
    WBD = {}
    _WC = {}

    def load_wc(wname):
        wc = cp.tile([D4, GS], FP32, name=f"wc_{wname}")
        nc.vector.memset(wc[:], 0.0)
        if wname == "fc67_w":
            nc.gpsimd.dma_start(wc[0:F, 0:1],
                                prm["fc6_w"][:, :].rearrange("o i -> i o"))
            nc.gpsimd.dma_start(wc[0:F, 1:3],
                                prm["fc7_w"][:, :].rearrange("o i -> i o"))
        else:
            o, i = prm[wname].shape
            nc.gpsimd.dma_start(wc[0:i, 0:o],
                                prm[wname][:, :].rearrange("o i -> i o"))
        _WC[wname] = wc

    def finish_weight(wname):
        wc = _WC[wname]
        tp = pp.tile([GS, 128], FP32, name=f"wt_{wname}", tag="b6",
                     padded_shape=[128, L])
        nc.tensor.matmul(tp[:], wc[:], rsel12[:])
        ts = sb.tile([GS, 128], FP32, name=f"ws_{wname}", tag="wts")
        nc.scalar.activation(ts[:], tp[:], AF.Copy)
        sp = pp.tile([128, 128], FP32, name=f"wsp_{wname}", tag="b7",
                     padded_shape=[128, L])
        nc.tensor.matmul(sp[:], ts[:], rsel16[:])
        wt = cp.tile([128, 128], FP32, name=f"W_{wname}")
        nc.vector.tensor_tensor(r(wt[:]), sp[:], mask_diag[:], OP.mult)
        WBD[wname] = wt

    load_wc("fc1_w")
    finish_weight("fc1_w")

    BIAS = {}
    _BCV = {}

    def load_bcv(bname):
        bcv = cp.tile([D4, 1], FP32, name=f"bcv_{bname}")
        nc.vector.memset(bcv[:], 0.0)
        if bname == "fc67_b":
            nc.gpsimd.dma_start(bcv[0:1, 0:1],
                                prm["fc6_b"][:].rearrange("(o u) -> o u", u=1))
            nc.gpsimd.dma_start(bcv[1:3, 0:1],
                                prm["fc7_b"][:].rearrange("(o u) -> o u", u=1))
        else:
            cnt = prm[bname].shape[0]
            nc.gpsimd.dma_start(bcv[0:cnt, 0:1],
                                prm[bname][:].rearrange("(o u) -> o u", u=1))
        _BCV[bname] = bcv

    def finish_bias(bname):
        bps = pp.tile([128, 1], FP32, name=f"bps_{bname}", tag="b2",
                      padded_shape=[128, L])
        nc.tensor.matmul(bps[:], rsel12[:], _BCV[bname][:])
        bt = cp.tile([128, 1], FP32, name=f"bias_{bname}")
        nc.scalar.activation(bt[:], bps[:], AF.Copy)
        BIAS[bname] = bt

    load_bcv("fc1_b")
    finish_bias("fc1_b")

    # bn scale/shift as [4, 512]: row j covers n in [512j, 512j+512)
    def bn_vec(name):
        t = cp.tile([4, L], FP32, name=f"v_{name}")
        nc.gpsimd.dma_start(t[:], prm[name][:].rearrange("(j t) -> j t", t=L))
        return t

    bng, bnb = {}, {}

    def load_bn_vecs(k):
        bng[k] = bn_vec(f"{k}_g")
        bnb[k] = bn_vec(f"{k}_b")

    load_bn_vecs("bn1")
    bnb_bc = {}

    def build_bnb_bc(k):
        bps = pp.tile([128, L], FP32, name=f"bnbps_{k}", tag="b3")
        nc.tensor.matmul(bps[:], bc4[:], bnb[k][:])
        bsb = cp.tile([128, L], FP32, name=f"bnbbc_{k}")
        nc.scalar.activation(bsb[:], bps[:], AF.Copy)
        bnb_bc[k] = bsb

    build_bnb_bc("bn1")

    # relmod scale a_r = wr*ps*ph/N as [128,1]
    a_r = []

    def emit_relmod_scales():
      for i in (1, 2, 3, 4):
        pst = sb.tile([1, 1], FP32, name=f"ps_{i}", tag="sc1")
        pht = sb.tile([1, 1], FP32, name=f"ph_{i}", tag="sc2")
        wrt = sb.tile([1, 1], FP32, name=f"wr_{i}", tag="sc3")
        nc.gpsimd.dma_start(pst[:], prm[f"ps{i}"][:].rearrange("(o u) -> o u", u=1))
        nc.gpsimd.dma_start(pht[:], prm[f"ph{i}"][:].rearrange("(o u) -> o u", u=1))
        nc.gpsimd.dma_start(wrt[:], prm[f"wr{i}"][:].rearrange("(o u) -> o u", u=1))
        nc.vector.tensor_tensor(pst[:], pst[:], pht[:], OP.mult)
        nc.vector.tensor_tensor(pst[:], pst[:], wrt[:], OP.mult)
        nc.vector.tensor_scalar_mul(pst[:], pst[:], 1.0 / N)
        pb = pp.tile([128, 1], FP32, name=f"psc_{i}", tag="b3",
                     padded_shape=[128, L])
        nc.tensor.matmul(pb[:], ones1[:], pst[:])
        at = cp.tile([128, 1], FP32, name=f"a_r{i}")
        nc.scalar.activation(at[:], pb[:], AF.Copy)
        a_r.append(at)

    # ================= helpers ===============================================
    def fc(w, src, name, plain=False):
        ps = pp.tile([128, L], FP32, name=f"psfc_{name}", tag="b0")
        if plain:
            nc.tensor.matmul(ps[:], w[:], src[:])
        else:
            mmr(ps[:], w[:], src[:])
        return ps

    def bn_send(h_ps, bias, tag):
        """fc PSUM -> biased hs + partial stats -> AllGather kickoff."""
        hs = sb.tile([128, L], FP32, name=f"hs_{tag}")
        nc.scalar.add(hs[:], h_ps[:], bias)
        sq = sb.tile([128, L], FP32, name=f"sq_{tag}")
        nc.scalar.activation(sq[:], h_ps[:], AF.Square, bias=bias)
        pk_s = pp.tile([4, L], FP32, name=f"pks_{tag}", tag="b1", padded_shape=[128, L])
        pk_q = pp.tile([4, L], FP32, name=f"pkq_{tag}", tag="b2", padded_shape=[128, L])
        nc.tensor.matmul(pk_s[:], onesfold[:], hs[:])
        nc.tensor.matmul(pk_q[:], onesfold[:], sq[:])
        sk_s = sb.tile([4, L], FP32, name=f"sks_{tag}")
        sk_q = sb.tile([4, L], FP32, name=f"skq_{tag}")
        nc.scalar.activation(sk_s[:], pk_s[:], AF.Copy)
        nc.vector.tensor_copy(sk_q[:], pk_q[:])
        cc_in = dr.tile([8, L], FP32, name=f"ccin_{tag}")
        cc_out = dr.tile([64, L], FP32, name=f"ccout_{tag}")
        nc.sync.dma_start(cc_in[0:4, :], sk_s[:])
        nc.scalar.dma_start(cc_in[4:8, :], sk_q[:])
        if single_core:
            # timing-only stand-in for the AllGather (TimelineSim path);
            # 4 serialized DMAs model the ~5us 8-core AllGather latency
            for r in range(4):
                nc.sync.dma_start(cc_out[8 * r:8 * r + 8, :], cc_in[:])
        else:
            nc.gpsimd.collective_compute(
                "AllGather",
                OP.bypass,
                replica_groups=[list(range(NCORES))],
                ins=[cc_in.opt()],
                outs=[cc_out.opt()],
            )
        return hs, cc_out

    def bn_recv(state, key, count_tile, tag):
        """Gathered stats -> bn(h) = a*(h-mean)+beta -> relu."""
        hs, cc_out = state
        gath = sb.tile([64, L], FP32, name=f"gath_{tag}")
        nc.sync.dma_start(gath[:], cc_out[:])
        m_ps = pp.tile([4, L], FP32, name=f"mps_{tag}", tag="b1", padded_shape=[128, L])
        q_ps = pp.tile([4, L], FP32, name=f"qps_{tag}", tag="b2", padded_shape=[128, L])
        nc.tensor.matmul(m_ps[:], count_tile[:, 0:4], gath[:])
        nc.tensor.matmul(q_ps[:], count_tile[:, 4:8], gath[:])
        mean = sb.tile([4, L], FP32, name=f"mean_{tag}")
        nc.scalar.activation(r(mean[:]), m_ps[:], AF.Copy)
        # h - mean (starts as soon as mean is up; off the rstd critical path)
        Mean_bc = pp.tile([128, L], FP32, name=f"Mbc_{tag}", tag="b4")
        mmr(Mean_bc[:], bc4r[:], mean[:])
        t1 = sb.tile([128, L], FP32, name=f"t1_{tag}")
        nc.vector.tensor_tensor(t1[:], hs[:], Mean_bc[:], OP.subtract)
        # a = gamma / sqrt(var+eps); Abs_reciprocal_sqrt is the one-op rstd
        # (ACT Rsqrt proper is banned; var+eps > 0 so abs is a no-op)
        msq = sb.tile([4, L], FP32, name=f"msq_{tag}")
        nc.scalar.activation(msq[:], m_ps[:], AF.Square)
        var = sb.tile([4, L], FP32, name=f"var_{tag}")
        nc.vector.tensor_tensor(var[:], q_ps[:], msq[:], OP.subtract)
        rstd = sb.tile([4, L], FP32, name=f"rstd_{tag}")
        if SAFE_RSTD:
            lv = sb.tile([4, L], FP32, name=f"lv_{tag}")
            nc.scalar.activation(lv[:], var[:], AF.Ln, bias=eps_t[0:4, :])
            nc.scalar.activation(rstd[:], lv[:], AF.Exp, scale=-0.5)
        else:
            nc.scalar.activation(rstd[:], var[:], AF.Abs_reciprocal_sqrt,
                                 bias=eps_t[0:4, :])
        a = sb.tile([4, L], FP32, name=f"a_{tag}")
        nc.vector.tensor_tensor(r(a[:]), rstd[:], bng[key][:], OP.mult)
        A_bc = pp.tile([128, L], FP32, name=f"Abc_{tag}", tag="b3")
        mmr(A_bc[:], bc4r[:], a[:])
        t2 = sb.tile([128, L], FP32, name=f"t2_{tag}")
        nc.vector.tensor_tensor(t2[:], t1[:], A_bc[:], OP.mult)
        t3 = sb.tile([128, L], FP32, name=f"t3_{tag}")
        nc.vector.tensor_tensor(t3[:], t2[:], bnb_bc[key][:], OP.add)
        hn = sb.tile([128, L], FP32, name=f"hn_{tag}")
        nc.vector.tensor_relu(r(hn[:]), t3[:])
        return hn

    def relmod(cur, wu, bu, at, idx):
        psU = pp.tile([128, L], FP32, name=f"psU_{idx}", tag="b0")
        mmr(psU[:], wu[:], cur[:])
        U = sb.tile([128, L], FP32, name=f"U_{idx}", tag="U")
        nc.scalar.activation(U[:], psU[:], AF.Relu, bias=bu)
        # s = sum_c cur^2 per token, broadcast to [128,L]
        sq = sb.tile([128, L], FP32, name=f"rsq_{idx}", tag="rsq")
        nc.scalar.activation(r(sq[:]), cur[:], AF.Square)
        psS = pp.tile([NG, L], FP32, name=f"psS_{idx}", tag="b5", padded_shape=[128, L])
        mmr(psS[:], ones_c16[:], sq[:])
        sS = sb.tile([NG, L], FP32, name=f"sS_{idx}", tag="sS")
        nc.vector.tensor_copy(r(sS[:]), psS[:])
        Sbc = pp.tile([128, L], FP32, name=f"Sbc_{idx}", tag="b3")
        mmr(Sbc[:], bc8r[:], sS[:])
        # transposes of cur and U (4x 128-chunks each)
        pTc = pp.tile([128, 4 * 128], FP32, name=f"pTc_{idx}", tag="b1")
        pTu = pp.tile([128, 4 * 128], FP32, name=f"pTu_{idx}", tag="b2")
        for j in range(4):
            nc.tensor.transpose(
                pTc[:, 128 * j:128 * (j + 1)], cur[:, 128 * j:128 * (j + 1)],
                ident128[:])
            nc.tensor.transpose(
                pTu[:, 128 * j:128 * (j + 1)], U[:, 128 * j:128 * (j + 1)],
                ident128[:])
        curT = sb.tile([128, 4 * 128], FP32, name=f"curT_{idx}", tag="curT")
        UT = sb.tile([128, 4 * 128], FP32, name=f"UT_{idx}", tag="UT")
        nc.scalar.activation(r(curT[:]), pTc[:], AF.Copy)
        nc.vector.tensor_copy(r(UT[:]), pTu[:])
        # P' = sum_t U x cur  (per-group partials on diag blocks)
        psG = pp.tile([128, 128], FP32, name=f"psG_{idx}", tag="b4",
                      padded_shape=[128, L])
        for j in range(4):
            mmr(psG[:], UT[:, 128 * j:128 * (j + 1)],
                curT[:, 128 * j:128 * (j + 1)],
                start=(j == 0), stop=(j == 3))
        Pm = sb.tile([128, 128], FP32, name=f"Pm_{idx}", tag="Pm")
        nc.vector.tensor_tensor(r(Pm[:]), psG[:], mask_diag[:], OP.mult)
        # G_spread = Phi^T (P_m Phi);  P_m = Pm^T
        psM = pp.tile([128, 128], FP32, name=f"psM_{idx}", tag="b5",
                      padded_shape=[128, L])
        mmr(psM[:], Pm[:], phi[:])
        Ms = sb.tile([128, 128], FP32, name=f"Ms_{idx}", tag="Ms")
        nc.scalar.activation(r(Ms[:]), psM[:], AF.Copy)
        psG2 = pp.tile([128, 128], FP32, name=f"psG2_{idx}", tag="b6",
                       padded_shape=[128, L])
        mmr(psG2[:], phi[:], Ms[:])
        Gf = sb.tile([128, 128], FP32, name=f"Gf_{idx}", tag="Gf")
        nc.vector.tensor_tensor(r(Gf[:]), psG2[:], mask_diag[:], OP.mult)
        # xG
        psXG = pp.tile([128, L], FP32, name=f"psXG_{idx}", tag="b6")
        mmr(psXG[:], Gf[:], cur[:])
        # out = (xG - s*U)*a + cur
        sbc_s = sb.tile([128, L], FP32, name=f"sbcs_{idx}", tag="sbcs")
        nc.scalar.activation(sbc_s[:], Sbc[:], AF.Copy)
        w1 = sb.tile([128, L], FP32, name=f"w1_{idx}", tag="w1")
        nc.gpsimd.tensor_tensor(w1[:], sbc_s[:], U[:], OP.mult)
        w2 = sb.tile([128, L], FP32, name=f"w2_{idx}", tag="w2")
        nc.vector.tensor_tensor(w2[:], psXG[:], w1[:], OP.subtract)
        nxt = sb.tile([128, L], FP32, name=f"nxt_{idx}", tag="nxt", bufs=2)
        nc.vector.scalar_tensor_tensor(
            r(nxt[:]), w2[:], at[:], cur[:], OP.mult, OP.add)
        return nxt

    # ================= network ===============================================
    st1 = bn_send(fc(WBD["fc1_w"], X, "1", plain=True), BIAS["fc1_b"][:], "bn1")
    # bn1 AllGather window: queue SWDGE loads + finish fc2/fc3/u1 params
    for w in ("fc2_w", "fc3_w", "u1_w"):
        load_wc(w)
    for b in ("fc2_b", "fc3_b", "u1_b"):
        load_bcv(b)
    load_bn_vecs("bn2")
    build_deferred_consts()
    finish_weight("fc2_w")
    finish_bias("fc2_b")
    finish_weight("fc3_w")
    finish_bias("fc3_b")
    finish_weight("u1_w")
    finish_bias("u1_b")
    build_bnb_bc("bn2")
    h1n = bn_recv(st1, "bn1", rr96, "bn1")
    st2 = bn_send(fc(WBD["fc2_w"], h1n, "2"), BIAS["fc2_b"][:], "bn2")
    # bn2 AllGather window: SWDGE loads first, then u-relmod param finishes
    # (their DMAs land mid-window, before bn2's rank matmuls need the PE)
    emit_relmod_scales()
    for w in ("u2_w", "u3_w", "u4_w"):
        load_wc(w)
    for b in ("u2_b", "u3_b", "u4_b"):
        load_bcv(b)
    for i in (2, 3, 4):
        finish_weight(f"u{i}_w")
        finish_bias(f"u{i}_b")
    h2n = bn_recv(st2, "bn2", rr192, "bn2")
    ps3 = fc(WBD["fc3_w"], h2n, "3")
    enc_r = sb.tile([128, L], FP32, name="enc_r")
    if SAFE_SIGMOID:
        nc.scalar.activation(enc_r[:], ps3[:], AF.Sigmoid, bias=BIAS["fc3_b"][:])
    else:
        # sigmoid(z) = 1/(1+exp(-z)) - keeps ACT on one table set
        b3neg = cp.tile([128, 1], FP32, name="b3neg")
        nc.vector.tensor_scalar_mul(b3neg[:], BIAS["fc3_b"][:], -1.0)
        ex = sb.tile([128, L], FP32, name="ex")
        nc.scalar.activation(ex[:], ps3[:], AF.Exp, scale=-1.0, bias=b3neg[:])
        exp1 = sb.tile([128, L], FP32, name="exp1")
        nc.vector.tensor_scalar_add(exp1[:], ex[:], 1.0)
        rec_scr = sb.tile([128, L], FP32, name="rec_scr")
        nc.vector.reciprocal_approx_accurate(enc_r[:], exp1[:], rec_scr[:])
    # zero the c>=12 garbage rows (sigmoid(0)=0.5) so downstream sums are clean
    enc = sb.tile([128, L], FP32, name="enc")
    nc.vector.tensor_scalar_mul(r(enc[:]), enc_r[:], colmask12[:])

    cur = enc
    for i in range(4):
        cur = relmod(cur, WBD[f"u{i + 1}_w"], BIAS[f"u{i + 1}_b"][:], a_r[i], i)
        if i == 0:
            for w in ("fc4_w", "fc5_w", "fc67_w"):
                load_wc(w)
            for b in ("fc4_b", "fc5_b", "fc67_b"):
                load_bcv(b)
            load_bn_vecs("bn4")
        elif i == 2:
            finish_weight("fc4_w")
            finish_bias("fc4_b")
            build_bnb_bc("bn4")

    st4 = bn_send(fc(WBD["fc4_w"], cur, "4"), BIAS["fc4_b"][:], "bn4")
    finish_weight("fc5_w")
    finish_bias("fc5_b")
    finish_weight("fc67_w")
    finish_bias("fc67_b")
    h4n = bn_recv(st4, "bn4", rr96, "bn4")
    ps5 = fc(WBD["fc5_w"], h4n, "5")
    h5 = sb.tile([128, L], FP32, name="h5")
    nc.scalar.activation(r(h5[:]), ps5[:], AF.Relu, bias=BIAS["fc5_b"][:])
    ps6 = fc(WBD["fc67_w"], h5, "6")
    outs = sb.tile([128, L], FP32, name="outs")
    nc.scalar.add(outs[:], ps6[:], BIAS["fc67_b"][:])

    for g in range(NG):
        b, n0 = g // 4, (g % 4) * L
        eng = nc.sync if g % 2 == 0 else nc.scalar
        eng.dma_start(out_d[b, n0:n0 + L, :].rearrange("n c -> c n"),
                      outs[GS * g:GS * g + F, :])


_PROGRAM = None


def _get_program():
    global _PROGRAM
    if _PROGRAM is None:
        _PROGRAM = _build()
    return _PROGRAM


def run(inputs, trace=False, **kw):
    inputs = {k: np.ascontiguousarray(np.asarray(v, np.float32))
              for k, v in inputs.items()}
    nc = _get_program()
    in_maps = []
    for i in range(NCORES):
        m = {name: inputs[name] for name, _ in PARAM_SPECS}
        m["x"] = np.ascontiguousarray(inputs["x"][BPC * i:BPC * (i + 1)])
        in_maps.append(m)
    last_exc = None
    for attempt in range(3):
        try:
            res = run_bass_kernel_spmd(
                nc, in_maps, core_ids=list(range(NCORES)), trace=trace, **kw)
            break
        except Exception as e:  # transient NRT_EXEC_UNIT_UNRECOVERABLE flakes
            last_exc = e
            import time
            time.sleep(5)
    else:
        raise last_exc
    out = np.concatenate([res.results[i]["out"] for i in range(NCORES)], axis=0)
    return out, res


def kernel(**inputs) -> np.ndarray:
    out, _ = run(inputs)
    return out



# revision 27
# speedup vs baseline: 1.2536x; 1.2536x over previous
"""Trainium2 Bass kernel for nn_Generator_34127810134219 (gnn_message_passing).

Strategy
--------
The reference relmod builds a [B,N,N] score matrix S = c*x@x^T (diag masked)
and computes wr*(S@U)/N + x.  Algebraically (verified to 4e-7 rel err):

    S@U = c*( x @ (x^T U) - ||x_i||^2 * U_i )

which collapses O(B*N^2*D) work into O(B*N*D^2).  The whole network is then a
memory-light pointwise/matmul pipeline over B*N = 32768 tokens with feature
dims <= 12.

Sharding: data-parallel over batch, 2 batches per core (8 cores).  The only
cross-core coupling is BatchNorm statistics (mean/var per n over batch and
feature dims) - exchanged as tiny [8,512] partial-sum tiles via AllGather
(3x), then reduced locally.  relmod is fully batch-local.

On-chip layout: feature-major, group-packed.  Per core 4096 tokens are split
into 8 groups of 512; group g lives on partitions [16g, 16g+C).  All fc
layers become single 128x512 matmuls with block-diagonal weights (float32r
for full-rate PE).  The per-batch Gram matrix G = x^T U is built with PE
transposes + matmuls; per-group partials are folded per batch as
mask . (Phi^T P_masked Phi) . mask with a fold matrix Phi - PE matmuls only,
no cross-partition vector ops.

Host-side prep (layout only, no arithmetic): x is fed as [BPC, F, N] so the
input/output DMAs are 2KB-contiguous; the tiny fc/unary weights are packed
into one [12,160] tile, biases into [12,10], relmod scalars into [12].  The
relmod scale a = wr*ps*ph/N is folded into the unary weights ON DEVICE
(relu(a*z) = a*relu(z), a >= 0), removing per-relmod scalar broadcasts.
"""

import numpy as np

import concourse.bass as bass
import concourse.bacc as bacc
import concourse.tile as tile
import concourse.mybir as mybir
from concourse.bass_utils import run_bass_kernel_spmd
from concourse.masks import make_identity

FP32 = mybir.dt.float32
F32R = mybir.dt.float32r
AF = mybir.ActivationFunctionType
OP = mybir.AluOpType

B, N, F = 16, 2048, 3
D2, D4 = 6, 12
NCORES = 8
BPC = B // NCORES          # batches per core
T = BPC * N                # tokens per core
NG = 8                     # groups per core
L = T // NG                # free-dim length (512)
GS = 16                    # partition stride per group
EPS = 1e-5

# weight slot order inside the packed [12,160] tile (each slot is 16 cols)
W_ORDER = ["fc1_w", "fc2_w", "fc3_w", "u1_w", "u2_w", "u3_w", "u4_w",
           "fc4_w", "fc5_w", "fc67_w"]
# (out, in) dims per slot (fc67 packed as fc6 col 0, fc7 cols 1:3)
W_DIMS = {"fc1_w": (D2, F), "fc2_w": (D4, D2), "fc3_w": (D4, D4),
          "u1_w": (D4, D4), "u2_w": (D4, D4), "u3_w": (D4, D4),
          "u4_w": (D4, D4), "fc4_w": (D2, D4), "fc5_w": (F, D2),
          "fc67_w": (F, F)}
WSLOT = {name: i for i, name in enumerate(W_ORDER)}

BN_VECS = ["bn1_g", "bn1_b", "bn2_g", "bn2_b", "bn4_g", "bn4_b"]


def _build(single_core=False):
    nc = bacc.Bacc(
        "TRN2",
        target_bir_lowering=False,
        debug=False,
        enable_asserts=False,
        num_devices=1 if single_core else NCORES,
    )

    x_d = nc.dram_tensor("x", [BPC, F, N], FP32, kind="ExternalInput")
    wall_d = nc.dram_tensor("wall", [D4, 16 * len(W_ORDER)], FP32,
                            kind="ExternalInput")
    ball_d = nc.dram_tensor("ball", [D4, len(W_ORDER)], FP32,
                            kind="ExternalInput")
    relsc_d = nc.dram_tensor("relsc", [12], FP32, kind="ExternalInput")
    prm = {name: nc.dram_tensor(name, [N], FP32, kind="ExternalInput")
           for name in BN_VECS}
    out_d = nc.dram_tensor("out", [BPC, F, N], FP32, kind="ExternalOutput")

    with tile.TileContext(nc) as tc:
        with (
            tc.tile_pool(name="consts", bufs=1) as cp,
            tc.tile_pool(name="sb", bufs=1) as sb,
            tc.tile_pool(name="pp", bufs=1, space="PSUM") as pp,
            tc.tile_pool(name="dram", bufs=1, space="DRAM") as dr,
        ):
            _emit(nc, tc, cp, sb, pp, dr, x_d, wall_d, ball_d, relsc_d,
                  prm, out_d, single_core=single_core)

    nc.compile()
    return nc


def _emit(nc, tc, cp, sb, pp, dr, x_d, wall_d, ball_d, relsc_d, prm, out_d,
          single_core=False):
    def mmr(out, lhsT, rhs, **kw):
        """float32r matmul: full-rate PE for fp32 bits (reduced mult precision)."""
        nc.tensor.matmul(out, lhsT.bitcast(F32R), rhs.bitcast(F32R), **kw)

    def r(ap):
        """f32r view for producer outputs feeding f32r matmuls (rounds)."""
        return ap.bitcast(F32R)

    eps_t = cp.tile([128, 1], FP32, name="eps_t")
    nc.gpsimd.memset(eps_t[:], EPS)
    # first ACT instruction resolves the table set once for the whole kernel
    actwarm = sb.tile([1, 1], FP32, name="actwarm")
    nc.scalar.activation(actwarm[:], eps_t[0:1, :], AF.Abs_reciprocal_sqrt)

    # ================= input / params (HWDGE, contiguous) ====================
    X = sb.tile([128, L], FP32, name="X")
    nc.vector.memset(X[:], 0.0)
    wall = cp.tile([D4, 16 * len(W_ORDER)], FP32, name="wall")
    nc.sync.dma_start(wall[:], wall_d[:, :])
    for b in range(BPC):
        for c in range(F):
            eng = nc.sync if c % 2 == 0 else nc.scalar
            eng.dma_start(
                X[64 * b:64 * b + 64, :].rearrange("(q s) f -> q s f", q=4)[:, c:c + 1, :],
                x_d[b, c].rearrange("(q f) -> q f", q=4).rearrange("q f -> q () f"))
    ball = cp.tile([D4, len(W_ORDER)], FP32, name="ball")
    nc.scalar.dma_start(ball[:], ball_d[:, :])
    relsc = sb.tile([1, 12], FP32, name="relsc")
    nc.scalar.dma_start(relsc[:], relsc_d[:].rearrange("(u s) -> u s", u=1))

    # ================= affine-built base selectors (Pool engine) =============
    # (zero-fills on DVE so Pool's serial affine chain - which gates fc1 -
    # stays as short as possible)
    def affine_sel(t, pattern, cm):
        """t := 1.0 where cm*p + pattern.idx == 0 else 0."""
        nc.vector.memset(t, 0.0)
        nc.gpsimd.affine_select(
            out=t, in_=t, compare_op=OP.not_equal, fill=1.0,
            base=0, pattern=pattern, channel_multiplier=cm)

    # bc8[g, (g',c)] = [g'==g]
    bc8 = cp.tile([NG, 128], FP32, name="bc8")
    affine_sel(bc8[:].rearrange("p (g c) -> p g c", c=GS), [[1, NG], [0, GS]], -1)
    # bc4[j, (g,c)] = [g%4==j]
    bc4 = cp.tile([4, 128], FP32, name="bc4")
    affine_sel(bc4[:].rearrange("p (h j c) -> p h j c", j=4, c=GS),
               [[0, 2], [1, 4], [0, GS]], -1)
    # bcB[b, (g,c)] = [g//4==b]
    bcB = cp.tile([2, 128], FP32, name="bcB")
    affine_sel(bcB[:].rearrange("p (b j c) -> p b j c", j=4, c=GS),
               [[1, 2], [0, 4], [0, GS]], -1)
    # RepSel12[ci', (g,ci)] = [ci==ci'] (ci'<12)
    rsel12 = cp.tile([D4, 128], FP32, name="rsel12")
    affine_sel(rsel12[:].rearrange("p (g c) -> p g c", c=GS), [[0, NG], [1, GS]], -1)
    # RepSel16
    rsel16 = cp.tile([GS, 128], FP32, name="rsel16")
    affine_sel(rsel16[:].rearrange("p (g c) -> p g c", c=GS), [[0, NG], [1, GS]], -1)
    # s8m[j, (r,j')] = [j'==j]; s8q[j, (r,j')] = [j'==j+4]  (stat-row selectors)
    s8m = cp.tile([4, 64], FP32, name="s8m")
    affine_sel(s8m[:].rearrange("p (r j) -> p r j", j=8), [[0, 8], [1, 8]], -1)
    s8q = cp.tile([4, 64], FP32, name="s8q")
    nc.vector.memset(s8q[:], 0.0)
    nc.gpsimd.affine_select(
        out=s8q[:].rearrange("p (r j) -> p r j", j=8), in_=s8q[:].rearrange("p (r j) -> p r j", j=8),
        compare_op=OP.not_equal, fill=1.0,
        base=-4, pattern=[[0, 8], [1, 8]], channel_multiplier=-1)

    ident128 = cp.tile([128, 128], FP32, name="ident128")
    make_identity(nc, ident128[:])
    identr = cp.tile([128, 128], FP32, name="identr")
    nc.vector.tensor_copy(identr[:].bitcast(F32R), ident128[:])
    ones1 = cp.tile([1, 128], FP32, name="ones1")
    nc.gpsimd.memset(ones1[:], 1.0)

    # bn scale/shift as [4, 512] (SWDGE, after the selectors so Pool's affine
    # work - which gates fc1 - isn't stuck behind these slow DMAs)
    bnvec = {}
    for name in BN_VECS:
        t = cp.tile([4, L], FP32, name=f"v_{name}")
        nc.gpsimd.dma_start(t[:].bitcast(F32R),
                            prm[name][:].rearrange("(j t) -> j t", t=L).bitcast(F32R))
        bnvec[name] = t

    # ================= PE-derived constant tiles =============================
    # mask_diag[(g,c),(g',c')] = [g==g']
    mask_ps = pp.tile([128, 128], FP32, name="mask_ps", tag="b0", padded_shape=[128, L])
    nc.tensor.matmul(mask_ps[:], bc8[:], bc8[:])
    mask_diag = cp.tile([128, 128], FP32, name="mask_diag")
    nc.scalar.activation(mask_diag[:], mask_ps[:], AF.Copy)
    # onesfold [128,4] = bc4^T (needed by the first bn_send pack matmuls)
    of_ps = pp.tile([128, 4], FP32, name="of_ps", tag="b3", padded_shape=[128, L])
    nc.tensor.transpose(of_ps[:], bc4[:], ident128[0:4, 0:4])
    onesfold = cp.tile([128, 4], FP32, name="onesfold")
    nc.scalar.activation(r(onesfold[:]), of_ps[:], AF.Copy)
    # f32r-rounded copies of bc4/bc8 (mmr operands must have f32r producers)
    bc4r = cp.tile([4, 128], FP32, name="bc4r")
    nc.vector.tensor_copy(r(bc4r[:]), bc4[:])
    bc8r = cp.tile([NG, 128], FP32, name="bc8r")
    nc.vector.tensor_copy(r(bc8r[:]), bc8[:])

    # relmod scale a_i = wr_i*ps_i*ph_i/N, broadcast to [128,1]
    scm = sb.tile([1, 4], FP32, name="scm")
    nc.vector.tensor_tensor(scm[:], relsc[:, 0:4], relsc[:, 4:8], OP.mult)
    nc.vector.tensor_tensor(scm[:], scm[:], relsc[:, 8:12], OP.mult)
    nc.vector.tensor_scalar_mul(scm[:], scm[:], 1.0 / N)
    a_r = []
    for i in range(4):
        pb = pp.tile([128, 1], FP32, name=f"psc_{i}", tag="b3",
                     padded_shape=[128, L])
        nc.tensor.matmul(pb[:], ones1[:], scm[:, i:i + 1])
        at = cp.tile([128, 1], FP32, name=f"a_r{i}")
        nc.scalar.activation(at[:], pb[:], AF.Copy)
        a_r.append(at)

    # deferred consts (phi/ones_c16/colmask12 and crep helpers) - emitted in
    # the bn1 AllGather window so they don't sit ahead of fc1 in the PE queue
    phi = cp.tile([128, 128], FP32, name="phi")
    ones_c16 = cp.tile([128, NG], FP32, name="ones_c16")
    colmask12 = cp.tile([128, 1], FP32, name="colmask12")
    ones12 = cp.tile([D4, 1], FP32, name="ones12")
    nc.gpsimd.memset(ones12[:], 1.0)

    # fused fold+broadcast matrices for bn stats: CM[k=(core,row), p] picks the
    # sum (CQ: sumsq) row of the gathered stats matching p's quarter, scaled by
    # 1/count, so mean/E[x2] land broadcast on all 128 partitions in ONE matmul
    CM, CQ = {}, {}

    def build_deferred_consts():
        crep_ps = pp.tile([128, 128], FP32, name="crep_ps", tag="b1",
                          padded_shape=[128, L])
        nc.tensor.matmul(crep_ps[:], rsel16[:], rsel16[:])
        crep = sb.tile([128, 128], FP32, name="crep")
        nc.scalar.activation(crep[:], crep_ps[:], AF.Copy)
        bmask_ps = pp.tile([128, 128], FP32, name="bmask_ps", tag="b2",
                           padded_shape=[128, L])
        nc.tensor.matmul(bmask_ps[:], bcB[:], bcB[:])
        nc.vector.tensor_tensor(r(phi[:]), bmask_ps[:], crep[:], OP.mult)
        oc_ps = pp.tile([128, NG], FP32, name="oc_ps", tag="b4",
                        padded_shape=[128, L])
        nc.tensor.transpose(oc_ps[:], bc8[:], ident128[0:NG, 0:NG])
        nc.scalar.activation(r(ones_c16[:]), oc_ps[:], AF.Copy)
        cm_ps = pp.tile([128, 1], FP32, name="cm_ps", tag="b5",
                        padded_shape=[128, L])
        nc.tensor.matmul(cm_ps[:], rsel12[:], ones12[:])
        nc.scalar.activation(colmask12[:], cm_ps[:], AF.Copy)
        for cnt in (96, 192):
            cmp_ = pp.tile([64, 128], FP32, name=f"cmps_{cnt}", tag="b6",
                           padded_shape=[128, L])
            nc.tensor.matmul(cmp_[:], s8m[:], bc4[:])
            cm = cp.tile([64, 128], FP32, name=f"CM_{cnt}")
            nc.scalar.activation(r(cm[:]), cmp_[:], AF.Copy, scale=1.0 / cnt)
            CM[cnt] = cm
            cqp = pp.tile([64, 128], FP32, name=f"cqps_{cnt}", tag="b7",
                          padded_shape=[128, L])
            nc.tensor.matmul(cqp[:], s8q[:], bc4[:])
            cq = cp.tile([64, 128], FP32, name=f"CQ_{cnt}")
            nc.scalar.activation(r(cq[:]), cqp[:], AF.Copy, scale=1.0 / cnt)
            CQ[cnt] = cq

    # ================= weights / biases ======================================
    # slot s of the packed wall tile holds W^T zero-padded to [12,16];
    # tp = Wc^T.rsel12 replicates the transpose across groups; sp = tp^T.rsel16
    # spreads along free; masking leaves the block-diagonal lhsT.  u-weights
    # are scaled by a_i here (relu(a z) = a relu(z)).
    WBD = {}
    BIAS = {}

    def finish_weight(wname, scale=None):
        s = WSLOT[wname]
        tp = pp.tile([GS, 128], FP32, name=f"wt_{wname}", tag="b6",
                     padded_shape=[128, L])
        nc.tensor.matmul(tp[:], wall[:, 16 * s:16 * (s + 1)], rsel12[:])
        ts = sb.tile([GS, 128], FP32, name=f"ws_{wname}", tag="wts")
        nc.scalar.activation(ts[:], tp[:], AF.Copy)
        sp = pp.tile([128, 128], FP32, name=f"wsp_{wname}", tag="b7",
                     padded_shape=[128, L])
        nc.tensor.matmul(sp[:], ts[:], rsel16[:])
        wt = cp.tile([128, 128], FP32, name=f"W_{wname}")
        if scale is None:
            nc.vector.tensor_tensor(r(wt[:]), sp[:], mask_diag[:], OP.mult)
        else:
            nc.vector.scalar_tensor_tensor(
                r(wt[:]), sp[:], scale[:], mask_diag[:], OP.mult, OP.mult)
        WBD[wname] = wt

    def finish_bias(wname, scale=None):
        s = WSLOT[wname]
        bps = pp.tile([128, 1], FP32, name=f"bps_{wname}", tag="b2",
                      padded_shape=[128, L])
        nc.tensor.matmul(bps[:], rsel12[:], ball[:, s:s + 1])
        bt = cp.tile([128, 1], FP32, name=f"bias_{wname}")
        nc.scalar.activation(bt[:], bps[:], AF.Copy)
        if scale is not None:
            bts = cp.tile([128, 1], FP32, name=f"biass_{wname}")
            nc.vector.tensor_tensor(bts[:], bt[:], scale[:], OP.mult)
            bt = bts
        BIAS[wname] = bt

    finish_weight("fc1_w")
    finish_bias("fc1_w")

    bnb_bc, bng_bc = {}, {}

    def build_bn_bc(k):
        bps = pp.tile([128, L], FP32, name=f"bnbps_{k}", tag="b3")
        mmr(bps[:], bc4r[:], bnvec[f"{k}_b"][:])
        bsb = cp.tile([128, L], FP32, name=f"bnbbc_{k}")
        nc.scalar.activation(bsb[:], bps[:], AF.Copy)
        bnb_bc[k] = bsb
        gps = pp.tile([128, L], FP32, name=f"bngps_{k}", tag="b4")
        mmr(gps[:], bc4r[:], bnvec[f"{k}_g"][:])
        gsb = cp.tile([128, L], FP32, name=f"bngbc_{k}")
        nc.scalar.activation(r(gsb[:]), gps[:], AF.Copy)
        bng_bc[k] = gsb

    # ================= helpers ===============================================
    def fc(w, src, name, plain=False):
        ps = pp.tile([128, L], FP32, name=f"psfc_{name}", tag="b0")
        if plain:
            nc.tensor.matmul(ps[:], w[:], src[:])
        else:
            mmr(ps[:], w[:], src[:])
        return ps

    def bn_send(h_ps, bias, tag):
        """fc PSUM -> biased hs + partial stats -> AllGather kickoff."""
        hs = sb.tile([128, L], FP32, name=f"hs_{tag}")
        nc.scalar.add(r(hs[:]), h_ps[:], bias)
        sq = sb.tile([128, L], FP32, name=f"sq_{tag}")
        nc.scalar.activation(r(sq[:]), h_ps[:], AF.Square, bias=bias)
        pk_s = pp.tile([4, L], FP32, name=f"pks_{tag}", tag="b1", padded_shape=[128, L])
        pk_q = pp.tile([4, L], FP32, name=f"pkq_{tag}", tag="b2", padded_shape=[128, L])
        mmr(pk_s[:], onesfold[:], hs[:])
        mmr(pk_q[:], onesfold[:], sq[:])
        sk_s = sb.tile([4, L], FP32, name=f"sks_{tag}")
        sk_q = sb.tile([4, L], FP32, name=f"skq_{tag}")
        nc.scalar.activation(sk_s[:], pk_s[:], AF.Copy)
        nc.vector.tensor_copy(sk_q[:], pk_q[:])
        cc_in = dr.tile([8, L], FP32, name=f"ccin_{tag}")
        cc_out = dr.tile([64, L], FP32, name=f"ccout_{tag}")
        nc.sync.dma_start(cc_in[0:4, :], sk_s[:])
        nc.scalar.dma_start(cc_in[4:8, :], sk_q[:])
        if single_core:
            # timing-only stand-in for the AllGather (TimelineSim path);
            # 4 serialized DMAs model the ~5us 8-core AllGather latency
            for rr in range(4):
                nc.sync.dma_start(cc_out[8 * rr:8 * rr + 8, :], cc_in[:])
        else:
            nc.gpsimd.collective_compute(
                "AllGather",
                OP.bypass,
                replica_groups=[list(range(NCORES))],
                ins=[cc_in.opt()],
                outs=[cc_out.opt()],
            )
        return hs, cc_out

    def bn_recv(state, key, cnt, tag):
        """Gathered stats -> bn(h) = a*(h-mean)+beta -> relu.

        Stats are reduced AND broadcast to [128,L] in one matmul each via the
        fused CM/CQ matrices; the whole affine chain runs on broadcast tiles.
        """
        hs, cc_out = state
        gath = sb.tile([64, L], FP32, name=f"gath_{tag}")
        nc.sync.dma_start(gath[:].bitcast(F32R), cc_out[:].bitcast(F32R))
        M_bc = pp.tile([128, L], FP32, name=f"Mbc_{tag}", tag="b4")
        mmr(M_bc[:], CM[cnt][:], gath[:])
        Q_bc = pp.tile([128, L], FP32, name=f"Qbc_{tag}", tag="b1")
        mmr(Q_bc[:], CQ[cnt][:], gath[:])
        # msq/var first (critical path); Square on Act - only one PSUM operand
        # is allowed per DVE TensorTensor
        msq = sb.tile([128, L], FP32, name=f"msq_{tag}")
        nc.scalar.activation(msq[:], M_bc[:], AF.Square)
        var = sb.tile([128, L], FP32, name=f"var_{tag}")
        nc.vector.tensor_tensor(var[:], Q_bc[:], msq[:], OP.subtract)
        # h - mean runs during the Act rstd (off the critical path)
        t1 = sb.tile([128, L], FP32, name=f"t1_{tag}")
        nc.vector.tensor_tensor(t1[:], hs[:], M_bc[:], OP.subtract)
        # a = gamma / sqrt(var+eps); Abs_reciprocal_sqrt is the one-op rstd
        # (var+eps > 0 so abs is a no-op)
        rstd = sb.tile([128, L], FP32, name=f"rstd_{tag}")
        nc.scalar.activation(rstd[:], var[:], AF.Abs_reciprocal_sqrt,
                             bias=eps_t[:])
        a = sb.tile([128, L], FP32, name=f"a_{tag}")
        nc.vector.tensor_tensor(a[:], rstd[:], bng_bc[key][:], OP.mult)
        t2 = sb.tile([128, L], FP32, name=f"t2_{tag}")
        nc.vector.tensor_tensor(t2[:], t1[:], a[:], OP.mult)
        t3 = sb.tile([128, L], FP32, name=f"t3_{tag}")
        nc.vector.tensor_tensor(t3[:], t2[:], bnb_bc[key][:], OP.add)
        hn = sb.tile([128, L], FP32, name=f"hn_{tag}")
        nc.vector.tensor_relu(r(hn[:]), t3[:])
        return hn

    def relmod(cur, wu, bu, idx):
        # U' = a*relu(unary(cur)) via the pre-scaled wu/bu
        psU = pp.tile([128, L], FP32, name=f"psU_{idx}", tag="b0")
        mmr(psU[:], wu[:], cur[:])
        # open the xG accumulator early with the +cur identity term so the
        # final output needs only ONE more matmul (Gf) and ONE vector op
        psXG = pp.tile([128, L], FP32, name=f"psXG_{idx}", tag="b7")
        nc.tensor.matmul(psXG[:], identr[:].bitcast(F32R), cur[:].bitcast(F32R), start=True, stop=False)
        U = sb.tile([128, L], FP32, name=f"U_{idx}", tag="U")
        nc.scalar.activation(r(U[:]), psU[:], AF.Relu, bias=bu)
        # s = sum_c cur^2 per token, broadcast to [128,L]
        sq = sb.tile([128, L], FP32, name=f"rsq_{idx}", tag="rsq")
        nc.scalar.activation(r(sq[:]), cur[:], AF.Square)
        psS = pp.tile([NG, L], FP32, name=f"psS_{idx}", tag="b5", padded_shape=[128, L])
        mmr(psS[:], ones_c16[:], sq[:])
        sS = sb.tile([NG, L], FP32, name=f"sS_{idx}", tag="sS")
        nc.vector.tensor_copy(r(sS[:]), psS[:])
        Sbc = pp.tile([128, L], FP32, name=f"Sbc_{idx}", tag="b3")
        mmr(Sbc[:], bc8r[:], sS[:])
        # transposes of cur and U (4x 128-chunks each, f32r for 1.5cyc/row)
        pTc = pp.tile([128, 4 * 128], FP32, name=f"pTc_{idx}", tag="b1")
        pTu = pp.tile([128, 4 * 128], FP32, name=f"pTu_{idx}", tag="b2")
        for j in range(4):
            nc.tensor.transpose(
                pTc[:, 128 * j:128 * (j + 1)].bitcast(F32R),
                cur[:, 128 * j:128 * (j + 1)].bitcast(F32R),
                identr[:].bitcast(F32R))
            nc.tensor.transpose(
                pTu[:, 128 * j:128 * (j + 1)].bitcast(F32R),
                U[:, 128 * j:128 * (j + 1)].bitcast(F32R),
                identr[:].bitcast(F32R))
        curT = sb.tile([128, 4 * 128], FP32, name=f"curT_{idx}", tag="curT")
        nc.scalar.activation(r(curT[:]), pTc[:], AF.Copy)
        # UT copied in halves so psG's accumulation starts one hop earlier
        UTa = sb.tile([128, 256], FP32, name=f"UTa_{idx}", tag="UTa")
        UTb = sb.tile([128, 256], FP32, name=f"UTb_{idx}", tag="UTb")
        nc.vector.tensor_copy(r(UTa[:]), pTu[:, 0:256])
        nc.vector.tensor_copy(r(UTb[:]), pTu[:, 256:512])
        # P' = sum_t U x cur  (per-group partials on diag blocks)
        psG = pp.tile([128, 128], FP32, name=f"psG_{idx}", tag="b4",
                      padded_shape=[128, L])
        for j in range(4):
            ut = UTa if j < 2 else UTb
            mmr(psG[:], ut[:, 128 * (j % 2):128 * (j % 2 + 1)],
                curT[:, 128 * j:128 * (j + 1)],
                start=(j == 0), stop=(j == 3))
        Pm = sb.tile([128, 128], FP32, name=f"Pm_{idx}", tag="Pm")
        nc.vector.tensor_tensor(r(Pm[:]), psG[:], mask_diag[:], OP.mult)
        # G_spread = Phi^T (P_m Phi);  P_m = Pm^T
        psM = pp.tile([128, 128], FP32, name=f"psM_{idx}", tag="b5",
                      padded_shape=[128, L])
        mmr(psM[:], Pm[:], phi[:])
        Ms = sb.tile([128, 128], FP32, name=f"Ms_{idx}", tag="Ms")
        nc.scalar.activation(r(Ms[:]), psM[:], AF.Copy)
        psG2 = pp.tile([128, 128], FP32, name=f"psG2_{idx}", tag="b6",
                       padded_shape=[128, L])
        mmr(psG2[:], phi[:], Ms[:])
        Gf = sb.tile([128, 128], FP32, name=f"Gf_{idx}", tag="Gf")
        nc.vector.tensor_tensor(r(Gf[:]), psG2[:], mask_diag[:], OP.mult)
        # xG + cur lands in the open accumulator
        mmr(psXG[:], Gf[:], cur[:], start=False, stop=True)
        # out = (xG + cur) - s*U   (a already folded into U)
        w1 = sb.tile([128, L], FP32, name=f"w1_{idx}", tag="w1")
        nc.vector.tensor_tensor(w1[:], Sbc[:], U[:], OP.mult)
        nxt = sb.tile([128, L], FP32, name=f"nxt_{idx}", tag="nxt", bufs=2)
        nc.vector.tensor_tensor(r(nxt[:]), psXG[:], w1[:], OP.subtract)
        return nxt

    # ================= network ===============================================
    st1 = bn_send(fc(WBD["fc1_w"], X, "1", plain=True), BIAS["fc1_w"][:], "bn1")
    # bn1 AllGather window: finish fc2/fc3/u* params, deferred consts
    build_deferred_consts()
    finish_weight("fc2_w")
    finish_bias("fc2_w")
    finish_weight("fc3_w")
    finish_bias("fc3_w")
    for i in range(4):
        finish_weight(f"u{i + 1}_w", scale=a_r[i])
        finish_bias(f"u{i + 1}_w", scale=a_r[i])
    build_bn_bc("bn1")
    build_bn_bc("bn2")
    h1n = bn_recv(st1, "bn1", 96, "bn1")
    st2 = bn_send(fc(WBD["fc2_w"], h1n, "2"), BIAS["fc2_w"][:], "bn2")
    # bn2 AllGather window: tail params
    finish_weight("fc4_w")
    finish_bias("fc4_w")
    finish_weight("fc5_w")
    finish_bias("fc5_w")
    finish_weight("fc67_w")
    finish_bias("fc67_w")
    build_bn_bc("bn4")
    h2n = bn_recv(st2, "bn2", 192, "bn2")
    ps3 = fc(WBD["fc3_w"], h2n, "3")
    enc_r = sb.tile([128, L], FP32, name="enc_r")
    nc.scalar.activation(enc_r[:], ps3[:], AF.Sigmoid, bias=BIAS["fc3_w"][:])
    # switch the Act tables back to the rsqrt set NOW (Act has slack during
    # the relmods) so bn4's rstd doesn't pay the 1.3us table load on-path
    actwarm2 = sb.tile([1, 1], FP32, name="actwarm2")
    nc.scalar.activation(actwarm2[:], eps_t[0:1, :], AF.Abs_reciprocal_sqrt)
    # zero the c>=12 garbage rows (sigmoid(0)=0.5) so downstream sums are clean
    enc = sb.tile([128, L], FP32, name="enc")
    nc.vector.tensor_scalar_mul(r(enc[:]), enc_r[:], colmask12[:])

    cur = enc
    for i in range(4):
        cur = relmod(cur, WBD[f"u{i + 1}_w"], BIAS[f"u{i + 1}_w"][:], i)

    st4 = bn_send(fc(WBD["fc4_w"], cur, "4"), BIAS["fc4_w"][:], "bn4")
    h4n = bn_recv(st4, "bn4", 96, "bn4")
    ps5 = fc(WBD["fc5_w"], h4n, "5")
    h5 = sb.tile([128, L], FP32, name="h5")
    nc.scalar.activation(r(h5[:]), ps5[:], AF.Relu, bias=BIAS["fc5_w"][:])
    ps6 = fc(WBD["fc67_w"], h5, "6")
    outs = sb.tile([128, L], FP32, name="outs")
    # per-batch bias add so each store DMA fires as soon as its half is ready
    for b in range(BPC):
        half = outs[64 * b:64 * b + 64, :]
        nc.scalar.add(half, ps6[64 * b:64 * b + 64, :], BIAS["fc67_w"][64 * b:64 * b + 64, :])
        for c in range(F):
            eng = nc.sync if c % 2 == 0 else nc.scalar
            eng.dma_start(
                out_d[b, c].rearrange("(q f) -> q f", q=4).rearrange("q f -> q () f"),
                half.rearrange("(q s) f -> q s f", q=4)[:, c:c + 1, :])


_PROGRAM = None


def _get_program():
    global _PROGRAM
    if _PROGRAM is None:
        _PROGRAM = _build()
    return _PROGRAM


def _pack_params(inputs):
    """Host-side LAYOUT-ONLY packing of the tiny weights (no arithmetic)."""
    wall = np.zeros((D4, 16 * len(W_ORDER)), np.float32)
    ball = np.zeros((D4, len(W_ORDER)), np.float32)
    for s, name in enumerate(W_ORDER):
        if name == "fc67_w":
            wall[0:F, 16 * s + 0:16 * s + 1] = inputs["fc6_w"].T
            wall[0:F, 16 * s + 1:16 * s + 3] = inputs["fc7_w"].T
            ball[0:1, s] = inputs["fc6_b"]
            ball[1:3, s] = inputs["fc7_b"]
        else:
            w = inputs[name]
            o, i = w.shape
            wall[0:i, 16 * s:16 * s + o] = w.T
            ball[0:o, s] = inputs[name.replace("_w", "_b")]
    relsc = np.concatenate([
        np.concatenate([inputs[f"ps{i}"] for i in range(1, 5)]),
        np.concatenate([inputs[f"ph{i}"] for i in range(1, 5)]),
        np.concatenate([inputs[f"wr{i}"] for i in range(1, 5)]),
    ]).astype(np.float32)
    return (np.ascontiguousarray(wall), np.ascontiguousarray(ball),
            np.ascontiguousarray(relsc))


def run(inputs, trace=False, **kw):
    inputs = {k: np.asarray(v, np.float32) for k, v in inputs.items()}
    nc = _get_program()
    wall, ball, relsc = _pack_params(inputs)
    base = {"wall": wall, "ball": ball, "relsc": relsc}
    for name in BN_VECS:
        base[name] = np.ascontiguousarray(inputs[name])
    in_maps = []
    for i in range(NCORES):
        m = dict(base)
        # feed x as [BPC, F, N] so the load DMA is contiguous (layout only)
        m["x"] = np.ascontiguousarray(
            inputs["x"][BPC * i:BPC * (i + 1)].transpose(0, 2, 1))
        in_maps.append(m)
    last_exc = None
    for attempt in range(3):
        try:
            res = run_bass_kernel_spmd(
                nc, in_maps, core_ids=list(range(NCORES)), trace=trace, **kw)
            break
        except Exception as e:  # transient NRT_EXEC_UNIT_UNRECOVERABLE flakes
            last_exc = e
            import time
            time.sleep(5)
    else:
        raise last_exc
    out = np.concatenate(
        [res.results[i]["out"].transpose(0, 2, 1) for i in range(NCORES)],
        axis=0)
    return np.ascontiguousarray(out), res


def kernel(**inputs) -> np.ndarray:
    out, _ = run(inputs)
    return out


# revision 30
# speedup vs baseline: 1.2690x; 1.0122x over previous
"""Trainium2 Bass kernel for nn_Generator_34127810134219 (gnn_message_passing).

Strategy
--------
The reference relmod builds a [B,N,N] score matrix S = c*x@x^T (diag masked)
and computes wr*(S@U)/N + x.  Algebraically (verified to 4e-7 rel err):

    S@U = c*( x @ (x^T U) - ||x_i||^2 * U_i )

which collapses O(B*N^2*D) work into O(B*N*D^2).  The whole network is then a
memory-light pointwise/matmul pipeline over B*N = 32768 tokens with feature
dims <= 12.

Sharding: data-parallel over batch, 2 batches per core (8 cores).  The only
cross-core coupling is BatchNorm statistics (mean/var per n over batch and
feature dims) - exchanged as tiny [8,512] partial-sum tiles via AllGather
(3x), then reduced locally.  relmod is fully batch-local.

On-chip layout: feature-major, group-packed.  Per core 4096 tokens are split
into 8 groups of 512; group g lives on partitions [16g, 16g+C).  All fc
layers become single 128x512 matmuls with block-diagonal weights (float32r
for full-rate PE).  The per-batch Gram matrix G = x^T U is built with PE
transposes + matmuls; per-group partials are folded per batch as
mask . (Phi^T P_masked Phi) . mask with a fold matrix Phi - PE matmuls only,
no cross-partition vector ops.

Host-side prep (layout only, no arithmetic): x is fed as [BPC, F, N] so the
input/output DMAs are 2KB-contiguous; the tiny fc/unary weights are packed
into one [12,160] tile, biases into [12,10], relmod scalars into [12].  The
relmod scale a = wr*ps*ph/N is folded into the unary weights ON DEVICE
(relu(a*z) = a*relu(z), a >= 0), removing per-relmod scalar broadcasts.
"""

import numpy as np

import concourse.bass as bass
import concourse.bacc as bacc
import concourse.tile as tile
import concourse.mybir as mybir
from concourse.bass_utils import run_bass_kernel_spmd
from concourse.masks import make_identity

FP32 = mybir.dt.float32
F32R = mybir.dt.float32r
AF = mybir.ActivationFunctionType
OP = mybir.AluOpType

B, N, F = 16, 2048, 3
D2, D4 = 6, 12
NCORES = 8
BPC = B // NCORES          # batches per core
T = BPC * N                # tokens per core
NG = 8                     # groups per core
L = T // NG                # free-dim length (512)
GS = 16                    # partition stride per group
EPS = 1e-5

# weight slot order inside the packed [12,160] tile (each slot is 16 cols)
W_ORDER = ["fc1_w", "fc2_w", "fc3_w", "u1_w", "u2_w", "u3_w", "u4_w",
           "fc4_w", "fc5_w", "fc67_w"]
# (out, in) dims per slot (fc67 packed as fc6 col 0, fc7 cols 1:3)
W_DIMS = {"fc1_w": (D2, F), "fc2_w": (D4, D2), "fc3_w": (D4, D4),
          "u1_w": (D4, D4), "u2_w": (D4, D4), "u3_w": (D4, D4),
          "u4_w": (D4, D4), "fc4_w": (D2, D4), "fc5_w": (F, D2),
          "fc67_w": (F, F)}
WSLOT = {name: i for i, name in enumerate(W_ORDER)}

BN_VECS = ["bn1_g", "bn1_b", "bn2_g", "bn2_b", "bn4_g", "bn4_b"]


def _build(single_core=False):
    nc = bacc.Bacc(
        "TRN2",
        target_bir_lowering=False,
        debug=False,
        enable_asserts=False,
        num_devices=1 if single_core else NCORES,
    )

    x_d = nc.dram_tensor("x", [BPC, F, N], FP32, kind="ExternalInput")
    wall_d = nc.dram_tensor("wall", [D4, 16 * len(W_ORDER)], FP32,
                            kind="ExternalInput")
    ball_d = nc.dram_tensor("ball", [D4, len(W_ORDER)], FP32,
                            kind="ExternalInput")
    relsc_d = nc.dram_tensor("relsc", [12], FP32, kind="ExternalInput")
    prm = {name: nc.dram_tensor(name, [N], FP32, kind="ExternalInput")
           for name in BN_VECS}
    out_d = nc.dram_tensor("out", [BPC, F, N], FP32, kind="ExternalOutput")

    with tile.TileContext(nc) as tc:
        with (
            tc.tile_pool(name="consts", bufs=1) as cp,
            tc.tile_pool(name="sb", bufs=1) as sb,
            tc.tile_pool(name="pp", bufs=1, space="PSUM") as pp,
            tc.tile_pool(name="dram", bufs=1, space="DRAM") as dr,
        ):
            _emit(nc, tc, cp, sb, pp, dr, x_d, wall_d, ball_d, relsc_d,
                  prm, out_d, single_core=single_core)

    nc.compile()
    return nc


def _emit(nc, tc, cp, sb, pp, dr, x_d, wall_d, ball_d, relsc_d, prm, out_d,
          single_core=False):
    def mmr(out, lhsT, rhs, **kw):
        """float32r matmul: full-rate PE for fp32 bits (reduced mult precision)."""
        nc.tensor.matmul(out, lhsT.bitcast(F32R), rhs.bitcast(F32R), **kw)

    def r(ap):
        """f32r view for producer outputs feeding f32r matmuls (rounds)."""
        return ap.bitcast(F32R)

    eps_t = cp.tile([128, 1], FP32, name="eps_t")
    nc.gpsimd.memset(eps_t[:], EPS)
    # first ACT instruction resolves the table set once for the whole kernel
    actwarm = sb.tile([1, 1], FP32, name="actwarm")
    nc.scalar.activation(actwarm[:], eps_t[0:1, :], AF.Abs_reciprocal_sqrt)

    # ================= input / params (HWDGE, contiguous) ====================
    X = sb.tile([128, L], FP32, name="X")
    nc.vector.memset(X[:], 0.0)
    wall = cp.tile([D4, 16 * len(W_ORDER)], FP32, name="wall")
    nc.sync.dma_start(wall[:], wall_d[:, :])
    for b in range(BPC):
        for c in range(F):
            eng = nc.sync if c % 2 == 0 else nc.scalar
            eng.dma_start(
                X[64 * b:64 * b + 64, :].rearrange("(q s) f -> q s f", q=4)[:, c:c + 1, :],
                x_d[b, c].rearrange("(q f) -> q f", q=4).rearrange("q f -> q () f"))
    ball = cp.tile([D4, len(W_ORDER)], FP32, name="ball")
    nc.scalar.dma_start(ball[:], ball_d[:, :])
    relsc = sb.tile([1, 12], FP32, name="relsc")
    nc.scalar.dma_start(relsc[:], relsc_d[:].rearrange("(u s) -> u s", u=1))

    # ================= affine-built base selectors (Pool engine) =============
    # (zero-fills on DVE so Pool's serial affine chain - which gates fc1 -
    # stays as short as possible)
    def affine_sel(t, pattern, cm):
        """t := 1.0 where cm*p + pattern.idx == 0 else 0."""
        nc.vector.memset(t, 0.0)
        nc.gpsimd.affine_select(
            out=t, in_=t, compare_op=OP.not_equal, fill=1.0,
            base=0, pattern=pattern, channel_multiplier=cm)

    # bc8[g, (g',c)] = [g'==g]
    bc8 = cp.tile([NG, 128], FP32, name="bc8")
    affine_sel(bc8[:].rearrange("p (g c) -> p g c", c=GS), [[1, NG], [0, GS]], -1)
    # bc4[j, (g,c)] = [g%4==j]
    bc4 = cp.tile([4, 128], FP32, name="bc4")
    affine_sel(bc4[:].rearrange("p (h j c) -> p h j c", j=4, c=GS),
               [[0, 2], [1, 4], [0, GS]], -1)
    # bcB[b, (g,c)] = [g//4==b]
    bcB = cp.tile([2, 128], FP32, name="bcB")
    affine_sel(bcB[:].rearrange("p (b j c) -> p b j c", j=4, c=GS),
               [[1, 2], [0, 4], [0, GS]], -1)
    # RepSel12[ci', (g,ci)] = [ci==ci'] (ci'<12)
    rsel12 = cp.tile([D4, 128], FP32, name="rsel12")
    affine_sel(rsel12[:].rearrange("p (g c) -> p g c", c=GS), [[0, NG], [1, GS]], -1)
    # RepSel16
    rsel16 = cp.tile([GS, 128], FP32, name="rsel16")
    affine_sel(rsel16[:].rearrange("p (g c) -> p g c", c=GS), [[0, NG], [1, GS]], -1)
    # s8m[j, (r,j')] = [j'==j]; s8q[j, (r,j')] = [j'==j+4]  (stat-row selectors)
    s8m = cp.tile([4, 64], FP32, name="s8m")
    affine_sel(s8m[:].rearrange("p (r j) -> p r j", j=8), [[0, 8], [1, 8]], -1)
    s8q = cp.tile([4, 64], FP32, name="s8q")
    nc.vector.memset(s8q[:], 0.0)
    nc.gpsimd.affine_select(
        out=s8q[:].rearrange("p (r j) -> p r j", j=8), in_=s8q[:].rearrange("p (r j) -> p r j", j=8),
        compare_op=OP.not_equal, fill=1.0,
        base=-4, pattern=[[0, 8], [1, 8]], channel_multiplier=-1)

    ident128 = cp.tile([128, 128], FP32, name="ident128")
    make_identity(nc, ident128[:])
    identr = cp.tile([128, 128], FP32, name="identr")
    nc.vector.tensor_copy(identr[:].bitcast(F32R), ident128[:])
    ones1 = cp.tile([1, 128], FP32, name="ones1")
    nc.gpsimd.memset(ones1[:], 1.0)

    # bn scale/shift as [4, 512] (SWDGE, after the selectors so Pool's affine
    # work - which gates fc1 - isn't stuck behind these slow DMAs)
    bnvec = {}
    for name in BN_VECS:
        t = cp.tile([4, L], FP32, name=f"v_{name}")
        nc.gpsimd.dma_start(t[:].bitcast(F32R),
                            prm[name][:].rearrange("(j t) -> j t", t=L).bitcast(F32R))
        bnvec[name] = t

    # ================= PE-derived constant tiles =============================
    # mask_diag[(g,c),(g',c')] = [g==g']
    mask_ps = pp.tile([128, 128], FP32, name="mask_ps", tag="b0", padded_shape=[128, L])
    nc.tensor.matmul(mask_ps[:], bc8[:], bc8[:])
    mask_diag = cp.tile([128, 128], FP32, name="mask_diag")
    nc.scalar.activation(mask_diag[:], mask_ps[:], AF.Copy)
    # onesfold [128,4] = bc4^T (needed by the first bn_send pack matmuls)
    of_ps = pp.tile([128, 4], FP32, name="of_ps", tag="b3", padded_shape=[128, L])
    nc.tensor.transpose(of_ps[:], bc4[:], ident128[0:4, 0:4])
    onesfold = cp.tile([128, 4], FP32, name="onesfold")
    nc.scalar.activation(r(onesfold[:]), of_ps[:], AF.Copy)
    # f32r-rounded copies of bc4/bc8 (mmr operands must have f32r producers)
    bc4r = cp.tile([4, 128], FP32, name="bc4r")
    nc.vector.tensor_copy(r(bc4r[:]), bc4[:])
    bc8r = cp.tile([NG, 128], FP32, name="bc8r")
    nc.vector.tensor_copy(r(bc8r[:]), bc8[:])

    # relmod scale a_i = wr_i*ps_i*ph_i/N, broadcast to [128,1]
    scm = sb.tile([1, 4], FP32, name="scm")
    nc.vector.tensor_tensor(scm[:], relsc[:, 0:4], relsc[:, 4:8], OP.mult)
    nc.vector.tensor_tensor(scm[:], scm[:], relsc[:, 8:12], OP.mult)
    nc.vector.tensor_scalar_mul(scm[:], scm[:], 1.0 / N)
    a_r = []
    for i in range(4):
        pb = pp.tile([128, 1], FP32, name=f"psc_{i}", tag="b3",
                     padded_shape=[128, L])
        nc.tensor.matmul(pb[:], ones1[:], scm[:, i:i + 1])
        at = cp.tile([128, 1], FP32, name=f"a_r{i}")
        nc.scalar.activation(at[:], pb[:], AF.Copy)
        a_r.append(at)

    # deferred consts (phi/ones_c16/colmask12 and crep helpers) - emitted in
    # the bn1 AllGather window so they don't sit ahead of fc1 in the PE queue
    phi = cp.tile([128, 128], FP32, name="phi")
    ones_c16 = cp.tile([128, NG], FP32, name="ones_c16")
    colmask12 = cp.tile([128, 1], FP32, name="colmask12")
    ones12 = cp.tile([D4, 1], FP32, name="ones12")
    nc.gpsimd.memset(ones12[:], 1.0)

    # fused fold+broadcast matrices for bn stats: CM[k=(core,row), p] picks the
    # sum (CQ: sumsq) row of the gathered stats matching p's quarter, scaled by
    # 1/count, so mean/E[x2] land broadcast on all 128 partitions in ONE matmul
    CM, CQ = {}, {}

    def build_deferred_consts():
        crep_ps = pp.tile([128, 128], FP32, name="crep_ps", tag="b1",
                          padded_shape=[128, L])
        nc.tensor.matmul(crep_ps[:], rsel16[:], rsel16[:])
        crep = sb.tile([128, 128], FP32, name="crep")
        nc.scalar.activation(crep[:], crep_ps[:], AF.Copy)
        bmask_ps = pp.tile([128, 128], FP32, name="bmask_ps", tag="b2",
                           padded_shape=[128, L])
        nc.tensor.matmul(bmask_ps[:], bcB[:], bcB[:])
        nc.vector.tensor_tensor(r(phi[:]), bmask_ps[:], crep[:], OP.mult)
        oc_ps = pp.tile([128, NG], FP32, name="oc_ps", tag="b4",
                        padded_shape=[128, L])
        nc.tensor.transpose(oc_ps[:], bc8[:], ident128[0:NG, 0:NG])
        nc.scalar.activation(r(ones_c16[:]), oc_ps[:], AF.Copy)
        cm_ps = pp.tile([128, 1], FP32, name="cm_ps", tag="b5",
                        padded_shape=[128, L])
        nc.tensor.matmul(cm_ps[:], rsel12[:], ones12[:])
        nc.scalar.activation(colmask12[:], cm_ps[:], AF.Copy)
        for cnt in (96, 192):
            cmp_ = pp.tile([64, 128], FP32, name=f"cmps_{cnt}", tag="b6",
                           padded_shape=[128, L])
            nc.tensor.matmul(cmp_[:], s8m[:], bc4[:])
            cm = cp.tile([64, 128], FP32, name=f"CM_{cnt}")
            nc.scalar.activation(r(cm[:]), cmp_[:], AF.Copy, scale=1.0 / cnt)
            CM[cnt] = cm
            cqp = pp.tile([64, 128], FP32, name=f"cqps_{cnt}", tag="b7",
                          padded_shape=[128, L])
            nc.tensor.matmul(cqp[:], s8q[:], bc4[:])
            cq = cp.tile([64, 128], FP32, name=f"CQ_{cnt}")
            nc.scalar.activation(r(cq[:]), cqp[:], AF.Copy, scale=1.0 / cnt)
            CQ[cnt] = cq

    # ================= weights / biases ======================================
    # slot s of the packed wall tile holds W^T zero-padded to [12,16];
    # tp = Wc^T.rsel12 replicates the transpose across groups; sp = tp^T.rsel16
    # spreads along free; masking leaves the block-diagonal lhsT.  u-weights
    # are scaled by a_i here (relu(a z) = a relu(z)).
    WBD = {}
    BIAS = {}

    def finish_weight(wname, scale=None):
        s = WSLOT[wname]
        tp = pp.tile([GS, 128], FP32, name=f"wt_{wname}", tag="b6",
                     padded_shape=[128, L])
        nc.tensor.matmul(tp[:], wall[:, 16 * s:16 * (s + 1)], rsel12[:])
        ts = sb.tile([GS, 128], FP32, name=f"ws_{wname}", tag="wts")
        nc.scalar.activation(ts[:], tp[:], AF.Copy)
        sp = pp.tile([128, 128], FP32, name=f"wsp_{wname}", tag="b7",
                     padded_shape=[128, L])
        nc.tensor.matmul(sp[:], ts[:], rsel16[:])
        wt = cp.tile([128, 128], FP32, name=f"W_{wname}")
        if scale is None:
            nc.vector.tensor_tensor(r(wt[:]), sp[:], mask_diag[:], OP.mult)
        else:
            nc.vector.scalar_tensor_tensor(
                r(wt[:]), sp[:], scale[:], mask_diag[:], OP.mult, OP.mult)
        WBD[wname] = wt

    def finish_bias(wname, scale=None):
        s = WSLOT[wname]
        bps = pp.tile([128, 1], FP32, name=f"bps_{wname}", tag="b2",
                      padded_shape=[128, L])
        nc.tensor.matmul(bps[:], rsel12[:], ball[:, s:s + 1])
        bt = cp.tile([128, 1], FP32, name=f"bias_{wname}")
        nc.scalar.activation(bt[:], bps[:], AF.Copy)
        if scale is not None:
            bts = cp.tile([128, 1], FP32, name=f"biass_{wname}")
            nc.vector.tensor_tensor(bts[:], bt[:], scale[:], OP.mult)
            bt = bts
        BIAS[wname] = bt

    finish_weight("fc1_w")
    finish_bias("fc1_w")

    bnb_bc, bng_bc = {}, {}

    def build_bn_bc(k):
        bps = pp.tile([128, L], FP32, name=f"bnbps_{k}", tag="b3")
        mmr(bps[:], bc4r[:], bnvec[f"{k}_b"][:])
        bsb = cp.tile([128, L], FP32, name=f"bnbbc_{k}")
        nc.scalar.activation(bsb[:], bps[:], AF.Copy)
        bnb_bc[k] = bsb
        gps = pp.tile([128, L], FP32, name=f"bngps_{k}", tag="b4")
        mmr(gps[:], bc4r[:], bnvec[f"{k}_g"][:])
        gsb = cp.tile([128, L], FP32, name=f"bngbc_{k}")
        nc.scalar.activation(r(gsb[:]), gps[:], AF.Copy)
        bng_bc[k] = gsb

    # ================= helpers ===============================================
    def fc(w, src, name, plain=False):
        ps = pp.tile([128, L], FP32, name=f"psfc_{name}", tag="b0")
        if plain:
            nc.tensor.matmul(ps[:], w[:], src[:])
        else:
            mmr(ps[:], w[:], src[:])
        return ps

    def bn_send(h_ps, bias, tag):
        """fc PSUM -> biased hs + partial stats -> AllGather kickoff."""
        sq = sb.tile([128, L], FP32, name=f"sq_{tag}")
        nc.scalar.activation(r(sq[:]), h_ps[:], AF.Square, bias=bias)
        hs = sb.tile([128, L], FP32, name=f"hs_{tag}")
        nc.scalar.add(r(hs[:]), h_ps[:], bias)
        pk_s = pp.tile([4, L], FP32, name=f"pks_{tag}", tag="b1", padded_shape=[128, L])
        pk_q = pp.tile([4, L], FP32, name=f"pkq_{tag}", tag="b2", padded_shape=[128, L])
        mmr(pk_s[:], onesfold[:], hs[:])
        mmr(pk_q[:], onesfold[:], sq[:])
        sk_s = sb.tile([4, L], FP32, name=f"sks_{tag}")
        sk_q = sb.tile([4, L], FP32, name=f"skq_{tag}")
        nc.scalar.activation(sk_s[:], pk_s[:], AF.Copy)
        nc.vector.tensor_copy(sk_q[:], pk_q[:])
        cc_in = dr.tile([8, L], FP32, name=f"ccin_{tag}")
        cc_out = dr.tile([64, L], FP32, name=f"ccout_{tag}")
        nc.sync.dma_start(cc_in[4:8, :], sk_q[:])
        nc.scalar.dma_start(cc_in[0:4, :], sk_s[:])
        if single_core:
            # timing-only stand-in for the AllGather (TimelineSim path);
            # 4 serialized DMAs model the ~5us 8-core AllGather latency
            for rr in range(4):
                nc.sync.dma_start(cc_out[8 * rr:8 * rr + 8, :], cc_in[:])
        else:
            nc.gpsimd.collective_compute(
                "AllGather",
                OP.bypass,
                replica_groups=[list(range(NCORES))],
                ins=[cc_in.opt()],
                outs=[cc_out.opt()],
            )
        return hs, cc_out

    def bn_recv(state, key, cnt, tag):
        """Gathered stats -> bn(h) = a*(h-mean)+beta -> relu.

        Stats are reduced AND broadcast to [128,L] in one matmul each via the
        fused CM/CQ matrices; the whole affine chain runs on broadcast tiles.
        """
        hs, cc_out = state
        gath = sb.tile([64, L], FP32, name=f"gath_{tag}")
        nc.sync.dma_start(gath[:].bitcast(F32R), cc_out[:].bitcast(F32R))
        M_bc = pp.tile([128, L], FP32, name=f"Mbc_{tag}", tag="b4")
        mmr(M_bc[:], CM[cnt][:], gath[:])
        Q_bc = pp.tile([128, L], FP32, name=f"Qbc_{tag}", tag="b1")
        mmr(Q_bc[:], CQ[cnt][:], gath[:])
        # msq/var first (critical path); Square on Act - only one PSUM operand
        # is allowed per DVE TensorTensor
        msq = sb.tile([128, L], FP32, name=f"msq_{tag}")
        nc.scalar.activation(msq[:], M_bc[:], AF.Square)
        var = sb.tile([128, L], FP32, name=f"var_{tag}")
        nc.vector.tensor_tensor(var[:], Q_bc[:], msq[:], OP.subtract)
        # h - mean runs during the Act rstd (off the critical path)
        t1 = sb.tile([128, L], FP32, name=f"t1_{tag}")
        nc.vector.tensor_tensor(t1[:], hs[:], M_bc[:], OP.subtract)
        # a = gamma / sqrt(var+eps); Abs_reciprocal_sqrt is the one-op rstd
        # (var+eps > 0 so abs is a no-op)
        rstd = sb.tile([128, L], FP32, name=f"rstd_{tag}")
        nc.scalar.activation(rstd[:], var[:], AF.Abs_reciprocal_sqrt,
                             bias=eps_t[:])
        a = sb.tile([128, L], FP32, name=f"a_{tag}")
        nc.vector.tensor_tensor(a[:], rstd[:], bng_bc[key][:], OP.mult)
        t2 = sb.tile([128, L], FP32, name=f"t2_{tag}")
        nc.vector.tensor_tensor(t2[:], t1[:], a[:], OP.mult)
        t3 = sb.tile([128, L], FP32, name=f"t3_{tag}")
        nc.vector.tensor_tensor(t3[:], t2[:], bnb_bc[key][:], OP.add)
        hn = sb.tile([128, L], FP32, name=f"hn_{tag}")
        nc.vector.tensor_relu(r(hn[:]), t3[:])
        return hn

    def relmod(cur, wu, bu, idx):
        # U' = a*relu(unary(cur)) via the pre-scaled wu/bu
        psU = pp.tile([128, L], FP32, name=f"psU_{idx}", tag="b0")
        mmr(psU[:], wu[:], cur[:])
        # open the xG accumulator early with the +cur identity term so the
        # final output needs only ONE more matmul (Gf) and ONE vector op
        psXG = pp.tile([128, L], FP32, name=f"psXG_{idx}", tag="b7")
        nc.tensor.matmul(psXG[:], identr[:].bitcast(F32R), cur[:].bitcast(F32R), start=True, stop=False)
        U = sb.tile([128, L], FP32, name=f"U_{idx}", tag="U")
        nc.scalar.activation(r(U[:]), psU[:], AF.Relu, bias=bu)
        # s = sum_c cur^2 per token, broadcast to [128,L]
        sq = sb.tile([128, L], FP32, name=f"rsq_{idx}", tag="rsq")
        nc.scalar.activation(r(sq[:]), cur[:], AF.Square)
        psS = pp.tile([NG, L], FP32, name=f"psS_{idx}", tag="b5", padded_shape=[128, L])
        mmr(psS[:], ones_c16[:], sq[:])
        sS = sb.tile([NG, L], FP32, name=f"sS_{idx}", tag="sS")
        nc.vector.tensor_copy(r(sS[:]), psS[:])
        Sbc = pp.tile([128, L], FP32, name=f"Sbc_{idx}", tag="b3")
        mmr(Sbc[:], bc8r[:], sS[:])
        # transposes of cur and U (4x 128-chunks each, f32r for 1.5cyc/row)
        pTc = pp.tile([128, 4 * 128], FP32, name=f"pTc_{idx}", tag="b1")
        pTu = pp.tile([128, 4 * 128], FP32, name=f"pTu_{idx}", tag="b2")
        for j in range(4):
            nc.tensor.transpose(
                pTc[:, 128 * j:128 * (j + 1)].bitcast(F32R),
                cur[:, 128 * j:128 * (j + 1)].bitcast(F32R),
                identr[:].bitcast(F32R))
            nc.tensor.transpose(
                pTu[:, 128 * j:128 * (j + 1)].bitcast(F32R),
                U[:, 128 * j:128 * (j + 1)].bitcast(F32R),
                identr[:].bitcast(F32R))
        curT = sb.tile([128, 4 * 128], FP32, name=f"curT_{idx}", tag="curT")
        nc.scalar.activation(r(curT[:]), pTc[:], AF.Copy)
        # UT copied in halves so psG's accumulation starts one hop earlier
        UTa = sb.tile([128, 256], FP32, name=f"UTa_{idx}", tag="UTa")
        UTb = sb.tile([128, 256], FP32, name=f"UTb_{idx}", tag="UTb")
        nc.vector.tensor_copy(r(UTa[:]), pTu[:, 0:256])
        nc.vector.tensor_copy(r(UTb[:]), pTu[:, 256:512])
        # P' = sum_t U x cur  (per-group partials on diag blocks)
        psG = pp.tile([128, 128], FP32, name=f"psG_{idx}", tag="b4",
                      padded_shape=[128, L])
        for j in range(4):
            ut = UTa if j < 2 else UTb
            mmr(psG[:], ut[:, 128 * (j % 2):128 * (j % 2 + 1)],
                curT[:, 128 * j:128 * (j + 1)],
                start=(j == 0), stop=(j == 3))
        Pm = sb.tile([128, 128], FP32, name=f"Pm_{idx}", tag="Pm")
        nc.vector.tensor_tensor(r(Pm[:]), psG[:], mask_diag[:], OP.mult)
        # G_spread = Phi^T (P_m Phi);  P_m = Pm^T
        psM = pp.tile([128, 128], FP32, name=f"psM_{idx}", tag="b5",
                      padded_shape=[128, L])
        mmr(psM[:], Pm[:], phi[:])
        Ms = sb.tile([128, 128], FP32, name=f"Ms_{idx}", tag="Ms")
        nc.scalar.activation(r(Ms[:]), psM[:], AF.Copy)
        psG2 = pp.tile([128, 128], FP32, name=f"psG2_{idx}", tag="b6",
                       padded_shape=[128, L])
        mmr(psG2[:], phi[:], Ms[:])
        Gf = sb.tile([128, 128], FP32, name=f"Gf_{idx}", tag="Gf")
        nc.vector.tensor_tensor(r(Gf[:]), psG2[:], mask_diag[:], OP.mult)
        # xG + cur lands in the open accumulator
        mmr(psXG[:], Gf[:], cur[:], start=False, stop=True)
        # out = (xG + cur) - s*U   (a already folded into U)
        w1 = sb.tile([128, L], FP32, name=f"w1_{idx}", tag="w1")
        nc.vector.tensor_tensor(w1[:], Sbc[:], U[:], OP.mult)
        nxt = sb.tile([128, L], FP32, name=f"nxt_{idx}", tag="nxt", bufs=2)
        nc.vector.tensor_tensor(r(nxt[:]), psXG[:], w1[:], OP.subtract)
        return nxt

    # ================= network ===============================================
    st1 = bn_send(fc(WBD["fc1_w"], X, "1", plain=True), BIAS["fc1_w"][:], "bn1")
    # bn1 AllGather window: finish fc2/fc3/u* params, deferred consts
    build_deferred_consts()
    finish_weight("fc2_w")
    finish_bias("fc2_w")
    finish_weight("fc3_w")
    finish_bias("fc3_w")
    for i in range(4):
        finish_weight(f"u{i + 1}_w", scale=a_r[i])
        finish_bias(f"u{i + 1}_w", scale=a_r[i])
    build_bn_bc("bn1")
    build_bn_bc("bn2")
    h1n = bn_recv(st1, "bn1", 96, "bn1")
    st2 = bn_send(fc(WBD["fc2_w"], h1n, "2"), BIAS["fc2_w"][:], "bn2")
    # bn2 AllGather window: tail params
    finish_weight("fc4_w")
    finish_bias("fc4_w")
    finish_weight("fc5_w")
    finish_bias("fc5_w")
    finish_weight("fc67_w")
    finish_bias("fc67_w")
    build_bn_bc("bn4")
    h2n = bn_recv(st2, "bn2", 192, "bn2")
    ps3 = fc(WBD["fc3_w"], h2n, "3")
    enc_r = sb.tile([128, L], FP32, name="enc_r")
    nc.scalar.activation(enc_r[:], ps3[:], AF.Sigmoid, bias=BIAS["fc3_w"][:])
    # switch the Act tables back to the rsqrt set NOW (Act has slack during
    # the relmods) so bn4's rstd doesn't pay the 1.3us table load on-path
    actwarm2 = sb.tile([1, 1], FP32, name="actwarm2")
    nc.scalar.activation(actwarm2[:], enc_r[0:1, 0:1], AF.Abs_reciprocal_sqrt)
    # zero the c>=12 garbage rows (sigmoid(0)=0.5) so downstream sums are clean
    enc = sb.tile([128, L], FP32, name="enc")
    nc.vector.tensor_scalar_mul(r(enc[:]), enc_r[:], colmask12[:])

    cur = enc
    for i in range(4):
        cur = relmod(cur, WBD[f"u{i + 1}_w"], BIAS[f"u{i + 1}_w"][:], i)

    # b67[(b,c'),0] = fc67 bias per output channel (b-independent)
    selb = cp.tile([128, 6], FP32, name="selb")
    affine_sel(selb[:].rearrange("p (b c) -> p b c", c=3), [[0, 2], [1, 3]], -1)
    b67ps = pp.tile([6, 1], FP32, name="b67ps", tag="b2", padded_shape=[128, L])
    nc.tensor.matmul(b67ps[:], selb[:], BIAS["fc67_w"][:])
    b67 = cp.tile([6, 1], FP32, name="b67")
    nc.scalar.activation(b67[:], b67ps[:], AF.Copy)

    st4 = bn_send(fc(WBD["fc4_w"], cur, "4"), BIAS["fc4_w"][:], "bn4")
    h4n = bn_recv(st4, "bn4", 96, "bn4")
    ps5 = fc(WBD["fc5_w"], h4n, "5")
    h5 = sb.tile([128, L], FP32, name="h5")
    nc.scalar.activation(r(h5[:]), ps5[:], AF.Relu, bias=BIAS["fc5_w"][:])
    # fused fc67: per quarter q, contract with the column slice of W_fc67
    # whose outputs are rows {64b+16q+c'} - the result lands directly in the
    # DRAM [3,2048]-per-batch layout, so the store is 2 contiguous DMAs
    oraw = sb.tile([6, 4 * L], FP32, name="oraw")
    w67 = WBD["fc67_w"][:].rearrange("p (b rest) -> p b rest", b=2)
    w67q = []
    for q in range(4):
        t = cp.tile([128, 6], FP32, name=f"w67q_{q}")
        nc.vector.tensor_copy(r(t[:]), w67[:, :, 16 * q:16 * q + F])
        w67q.append(t)
    for q in range(4):
        psq = pp.tile([6, L], FP32, name=f"psraw_{q}", tag=f"b{4 + (q % 2)}",
                      padded_shape=[128, L])
        mmr(psq[:], w67q[q][:], h5[:])
        if q % 2 == 0:
            nc.scalar.add(oraw[:, L * q:L * (q + 1)], psq[:], b67[:])
        else:
            nc.vector.tensor_scalar_add(oraw[:, L * q:L * (q + 1)], psq[:], b67[:])
    for b in range(BPC):
        eng = nc.sync if b % 2 == 0 else nc.scalar
        eng.dma_start(
            out_d[b][:, :],
            oraw[F * b:F * b + F, :].rearrange("c (q f) -> c (q f)", q=4))


_PROGRAM = None


def _get_program():
    global _PROGRAM
    if _PROGRAM is None:
        _PROGRAM = _build()
    return _PROGRAM


def _pack_params(inputs):
    """Host-side LAYOUT-ONLY packing of the tiny weights (no arithmetic)."""
    wall = np.zeros((D4, 16 * len(W_ORDER)), np.float32)
    ball = np.zeros((D4, len(W_ORDER)), np.float32)
    for s, name in enumerate(W_ORDER):
        if name == "fc67_w":
            wall[0:F, 16 * s + 0:16 * s + 1] = inputs["fc6_w"].T
            wall[0:F, 16 * s + 1:16 * s + 3] = inputs["fc7_w"].T
            ball[0:1, s] = inputs["fc6_b"]
            ball[1:3, s] = inputs["fc7_b"]
        else:
            w = inputs[name]
            o, i = w.shape
            wall[0:i, 16 * s:16 * s + o] = w.T
            ball[0:o, s] = inputs[name.replace("_w", "_b")]
    relsc = np.concatenate([
        np.concatenate([inputs[f"ps{i}"] for i in range(1, 5)]),
        np.concatenate([inputs[f"ph{i}"] for i in range(1, 5)]),
        np.concatenate([inputs[f"wr{i}"] for i in range(1, 5)]),
    ]).astype(np.float32)
    return (np.ascontiguousarray(wall), np.ascontiguousarray(ball),
            np.ascontiguousarray(relsc))


def run(inputs, trace=False, **kw):
    inputs = {k: np.asarray(v, np.float32) for k, v in inputs.items()}
    nc = _get_program()
    wall, ball, relsc = _pack_params(inputs)
    base = {"wall": wall, "ball": ball, "relsc": relsc}
    for name in BN_VECS:
        base[name] = np.ascontiguousarray(inputs[name])
    in_maps = []
    for i in range(NCORES):
        m = dict(base)
        # feed x as [BPC, F, N] so the load DMA is contiguous (layout only)
        m["x"] = np.ascontiguousarray(
            inputs["x"][BPC * i:BPC * (i + 1)].transpose(0, 2, 1))
        in_maps.append(m)
    last_exc = None
    for attempt in range(3):
        try:
            res = run_bass_kernel_spmd(
                nc, in_maps, core_ids=list(range(NCORES)), trace=trace, **kw)
            break
        except Exception as e:  # transient NRT_EXEC_UNIT_UNRECOVERABLE flakes
            last_exc = e
            import time
            time.sleep(5)
    else:
        raise last_exc
    out = np.concatenate(
        [res.results[i]["out"].transpose(0, 2, 1) for i in range(NCORES)],
        axis=0)
    return np.ascontiguousarray(out), res


def kernel(**inputs) -> np.ndarray:
    out, _ = run(inputs)
    return out


# revision 36
# speedup vs baseline: 1.2937x; 1.0195x over previous
"""Trainium2 Bass kernel for nn_Generator_34127810134219 (gnn_message_passing).

Strategy
--------
The reference relmod builds a [B,N,N] score matrix S = c*x@x^T (diag masked)
and computes wr*(S@U)/N + x.  Algebraically (verified to 4e-7 rel err):

    S@U = c*( x @ (x^T U) - ||x_i||^2 * U_i )

which collapses O(B*N^2*D) work into O(B*N*D^2).  The whole network is then a
memory-light pointwise/matmul pipeline over B*N = 32768 tokens with feature
dims <= 12.

Sharding: data-parallel over batch, 2 batches per core (8 cores).  The only
cross-core coupling is BatchNorm statistics (mean/var per n over batch and
feature dims) - exchanged as tiny [8,512] partial-sum tiles via AllGather
(3x), then reduced locally.  relmod is fully batch-local.

On-chip layout: feature-major, group-packed.  Per core 4096 tokens are split
into 8 groups of 512; group g lives on partitions [16g, 16g+C).  All fc
layers become single 128x512 matmuls with block-diagonal weights (float32r
for full-rate PE).  The per-batch Gram matrix G = x^T U is built with PE
transposes + matmuls; per-group partials are folded per batch as
mask . (Phi^T P_masked Phi) . mask with a fold matrix Phi - PE matmuls only,
no cross-partition vector ops.

Host-side prep (layout only, no arithmetic): x is fed as [BPC, F, N] so the
input/output DMAs are 2KB-contiguous; the tiny fc/unary weights are packed
into one [12,160] tile, biases into [12,10], relmod scalars into [12].  The
relmod scale a = wr*ps*ph/N is folded into the unary weights ON DEVICE
(relu(a*z) = a*relu(z), a >= 0), removing per-relmod scalar broadcasts.
"""

import numpy as np

import concourse.bass as bass
import concourse.bacc as bacc
import concourse.tile as tile
import concourse.mybir as mybir
from concourse.bass_utils import run_bass_kernel_spmd
from concourse.masks import make_identity

FP32 = mybir.dt.float32
F32R = mybir.dt.float32r
AF = mybir.ActivationFunctionType
OP = mybir.AluOpType

B, N, F = 16, 2048, 3
D2, D4 = 6, 12
NCORES = 8
BPC = B // NCORES          # batches per core
T = BPC * N                # tokens per core
NG = 8                     # groups per core
L = T // NG                # free-dim length (512)
GS = 16                    # partition stride per group
EPS = 1e-5

# weight slot order inside the packed [12,160] tile (each slot is 16 cols)
W_ORDER = ["fc1_w", "fc2_w", "fc3_w", "u1_w", "u2_w", "u3_w", "u4_w",
           "fc4_w", "fc5_w", "fc67_w"]
# (out, in) dims per slot (fc67 packed as fc6 col 0, fc7 cols 1:3)
W_DIMS = {"fc1_w": (D2, F), "fc2_w": (D4, D2), "fc3_w": (D4, D4),
          "u1_w": (D4, D4), "u2_w": (D4, D4), "u3_w": (D4, D4),
          "u4_w": (D4, D4), "fc4_w": (D2, D4), "fc5_w": (F, D2),
          "fc67_w": (F, F)}
WSLOT = {name: i for i, name in enumerate(W_ORDER)}

BN_VECS = ["bn1_g", "bn1_b", "bn2_g", "bn2_b", "bn4_g", "bn4_b"]


def _build(single_core=False):
    nc = bacc.Bacc(
        "TRN2",
        target_bir_lowering=False,
        debug=False,
        enable_asserts=False,
        num_devices=1 if single_core else NCORES,
    )

    x_d = nc.dram_tensor("x", [BPC, F, N], FP32, kind="ExternalInput")
    wall_d = nc.dram_tensor("wall", [D4, 16 * len(W_ORDER)], FP32,
                            kind="ExternalInput")
    ball_d = nc.dram_tensor("ball", [D4, len(W_ORDER)], FP32,
                            kind="ExternalInput")
    relsc_d = nc.dram_tensor("relsc", [12], FP32, kind="ExternalInput")
    prm = {name: nc.dram_tensor(name, [N], FP32, kind="ExternalInput")
           for name in BN_VECS}
    out_d = nc.dram_tensor("out", [BPC, F, N], FP32, kind="ExternalOutput")

    with tile.TileContext(nc) as tc:
        with (
            tc.tile_pool(name="consts", bufs=1) as cp,
            tc.tile_pool(name="sb", bufs=1) as sb,
            tc.tile_pool(name="pp", bufs=1, space="PSUM") as pp,
            tc.tile_pool(name="dram", bufs=1, space="DRAM") as dr,
        ):
            _emit(nc, tc, cp, sb, pp, dr, x_d, wall_d, ball_d, relsc_d,
                  prm, out_d, single_core=single_core)

    nc.compile()
    return nc


def _emit(nc, tc, cp, sb, pp, dr, x_d, wall_d, ball_d, relsc_d, prm, out_d,
          single_core=False):
    def mmr(out, lhsT, rhs, **kw):
        """float32r matmul: full-rate PE for fp32 bits (reduced mult precision)."""
        nc.tensor.matmul(out, lhsT.bitcast(F32R), rhs.bitcast(F32R), **kw)

    def r(ap):
        """f32r view for producer outputs feeding f32r matmuls (rounds)."""
        return ap.bitcast(F32R)

    eps_t = cp.tile([128, 1], FP32, name="eps_t")
    nc.gpsimd.memset(eps_t[:], EPS)
    # first ACT instruction resolves the table set once for the whole kernel
    actwarm = sb.tile([1, 1], FP32, name="actwarm")
    nc.scalar.activation(actwarm[:], eps_t[0:1, :], AF.Abs_reciprocal_sqrt)

    # ================= input / params (HWDGE, contiguous) ====================
    X = sb.tile([128, L], FP32, name="X")
    nc.vector.memset(X[:], 0.0)
    wall = cp.tile([D4, 16 * len(W_ORDER)], FP32, name="wall")
    nc.sync.dma_start(wall[:], wall_d[:, :])
    for b in range(BPC):
        for c in range(F):
            eng = nc.sync if c % 2 == 0 else nc.scalar
            eng.dma_start(
                X[64 * b:64 * b + 64, :].rearrange("(q s) f -> q s f", q=4)[:, c:c + 1, :],
                x_d[b, c].rearrange("(q f) -> q f", q=4).rearrange("q f -> q () f"))
    ball = cp.tile([D4, len(W_ORDER)], FP32, name="ball")
    nc.scalar.dma_start(ball[:], ball_d[:, :])
    relsc = sb.tile([1, 12], FP32, name="relsc")
    nc.scalar.dma_start(relsc[:], relsc_d[:].rearrange("(u s) -> u s", u=1))

    # ================= affine-built base selectors (Pool engine) =============
    # (zero-fills on DVE so Pool's serial affine chain - which gates fc1 -
    # stays as short as possible)
    def affine_sel(t, pattern, cm):
        """t := 1.0 where cm*p + pattern.idx == 0 else 0."""
        nc.vector.memset(t, 0.0)
        nc.gpsimd.affine_select(
            out=t, in_=t, compare_op=OP.not_equal, fill=1.0,
            base=0, pattern=pattern, channel_multiplier=cm)

    # bc8[g, (g',c)] = [g'==g]
    bc8 = cp.tile([NG, 128], FP32, name="bc8")
    affine_sel(bc8[:].rearrange("p (g c) -> p g c", c=GS), [[1, NG], [0, GS]], -1)
    # bc4[j, (g,c)] = [g%4==j]
    bc4 = cp.tile([4, 128], FP32, name="bc4")
    affine_sel(bc4[:].rearrange("p (h j c) -> p h j c", j=4, c=GS),
               [[0, 2], [1, 4], [0, GS]], -1)
    # bcB[b, (g,c)] = [g//4==b]
    bcB = cp.tile([2, 128], FP32, name="bcB")
    affine_sel(bcB[:].rearrange("p (b j c) -> p b j c", j=4, c=GS),
               [[1, 2], [0, 4], [0, GS]], -1)
    # RepSel12[ci', (g,ci)] = [ci==ci'] (ci'<12)
    rsel12 = cp.tile([D4, 128], FP32, name="rsel12")
    affine_sel(rsel12[:].rearrange("p (g c) -> p g c", c=GS), [[0, NG], [1, GS]], -1)
    # RepSel16
    rsel16 = cp.tile([GS, 128], FP32, name="rsel16")
    affine_sel(rsel16[:].rearrange("p (g c) -> p g c", c=GS), [[0, NG], [1, GS]], -1)
    # s8m[j, (r,j')] = [j'==j]; s8q[j, (r,j')] = [j'==j+4]  (stat-row selectors)
    s8m = cp.tile([4, 64], FP32, name="s8m")
    affine_sel(s8m[:].rearrange("p (r j) -> p r j", j=8), [[0, 8], [1, 8]], -1)
    s8q = cp.tile([4, 64], FP32, name="s8q")
    nc.vector.memset(s8q[:], 0.0)
    nc.gpsimd.affine_select(
        out=s8q[:].rearrange("p (r j) -> p r j", j=8), in_=s8q[:].rearrange("p (r j) -> p r j", j=8),
        compare_op=OP.not_equal, fill=1.0,
        base=-4, pattern=[[0, 8], [1, 8]], channel_multiplier=-1)

    ident128 = cp.tile([128, 128], FP32, name="ident128")
    make_identity(nc, ident128[:])
    identr = cp.tile([128, 128], FP32, name="identr")
    nc.vector.tensor_copy(identr[:].bitcast(F32R), ident128[:])
    ones1 = cp.tile([1, 128], FP32, name="ones1")
    nc.gpsimd.memset(ones1[:], 1.0)

    # bn scale/shift as [4, 512] (SWDGE, after the selectors so Pool's affine
    # work - which gates fc1 - isn't stuck behind these slow DMAs)
    bnvec = {}
    for name in BN_VECS:
        t = cp.tile([4, L], FP32, name=f"v_{name}")
        nc.gpsimd.dma_start(t[:].bitcast(F32R),
                            prm[name][:].rearrange("(j t) -> j t", t=L).bitcast(F32R))
        bnvec[name] = t

    # ================= PE-derived constant tiles =============================
    # mask_diag[(g,c),(g',c')] = [g==g']
    mask_ps = pp.tile([128, 128], FP32, name="mask_ps", tag="b0", padded_shape=[128, L])
    nc.tensor.matmul(mask_ps[:], bc8[:], bc8[:])
    mask_diag = cp.tile([128, 128], FP32, name="mask_diag")
    nc.scalar.activation(mask_diag[:], mask_ps[:], AF.Copy)
    # onesfold [128,4] = bc4^T (needed by the first bn_send pack matmuls)
    of_ps = pp.tile([128, 4], FP32, name="of_ps", tag="b3", padded_shape=[128, L])
    nc.tensor.transpose(of_ps[:], bc4[:], ident128[0:4, 0:4])
    onesfold = cp.tile([128, 4], FP32, name="onesfold")
    nc.scalar.activation(r(onesfold[:]), of_ps[:], AF.Copy)
    # f32r-rounded copies of selectors (mmr operands must have f32r producers)
    rsel12r = cp.tile([D4, 128], FP32, name="rsel12r")
    nc.vector.tensor_copy(rsel12r[:].bitcast(F32R), rsel12[:])
    bc4r = cp.tile([4, 128], FP32, name="bc4r")
    nc.vector.tensor_copy(r(bc4r[:]), bc4[:])
    bc8r = cp.tile([NG, 128], FP32, name="bc8r")
    nc.vector.tensor_copy(r(bc8r[:]), bc8[:])

    # relmod scale a_i = wr_i*ps_i*ph_i/N, broadcast to [128,1]
    scm = sb.tile([1, 4], FP32, name="scm")
    nc.vector.tensor_tensor(scm[:], relsc[:, 0:4], relsc[:, 4:8], OP.mult)
    nc.vector.tensor_tensor(scm[:], scm[:], relsc[:, 8:12], OP.mult)
    nc.vector.tensor_scalar_mul(scm[:], scm[:], 1.0 / N)
    a_r = []
    for i in range(4):
        pb = pp.tile([128, 1], FP32, name=f"psc_{i}", tag="b3",
                     padded_shape=[128, L])
        nc.tensor.matmul(pb[:], ones1[:], scm[:, i:i + 1])
        at = cp.tile([128, 1], FP32, name=f"a_r{i}")
        nc.scalar.activation(at[:], pb[:], AF.Copy)
        a_r.append(at)

    # deferred consts (phi/ones_c16/colmask12 and crep helpers) - emitted in
    # the bn1 AllGather window so they don't sit ahead of fc1 in the PE queue
    phi = cp.tile([128, 128], FP32, name="phi")
    ones_c16 = cp.tile([128, NG], FP32, name="ones_c16")
    colmask12 = cp.tile([128, 1], FP32, name="colmask12")
    ones12 = cp.tile([D4, 1], FP32, name="ones12")
    nc.gpsimd.memset(ones12[:], 1.0)

    # fused fold+broadcast matrices for bn stats: CM[k=(core,row), p] picks the
    # sum (CQ: sumsq) row of the gathered stats matching p's quarter, scaled by
    # 1/count, so mean/E[x2] land broadcast on all 128 partitions in ONE matmul
    CM, CQ = {}, {}

    def build_deferred_consts():
        crep_ps = pp.tile([128, 128], FP32, name="crep_ps", tag="b1",
                          padded_shape=[128, L])
        nc.tensor.matmul(crep_ps[:], rsel16[:], rsel16[:])
        crep = sb.tile([128, 128], FP32, name="crep")
        nc.scalar.activation(crep[:], crep_ps[:], AF.Copy)
        bmask_ps = pp.tile([128, 128], FP32, name="bmask_ps", tag="b2",
                           padded_shape=[128, L])
        nc.tensor.matmul(bmask_ps[:], bcB[:], bcB[:])
        nc.vector.tensor_tensor(r(phi[:]), bmask_ps[:], crep[:], OP.mult)
        oc_ps = pp.tile([128, NG], FP32, name="oc_ps", tag="b4",
                        padded_shape=[128, L])
        nc.tensor.transpose(oc_ps[:], bc8[:], ident128[0:NG, 0:NG])
        nc.scalar.activation(r(ones_c16[:]), oc_ps[:], AF.Copy)
        cm_ps = pp.tile([128, 1], FP32, name="cm_ps", tag="b5",
                        padded_shape=[128, L])
        nc.tensor.matmul(cm_ps[:], rsel12[:], ones12[:])
        nc.scalar.activation(colmask12[:], cm_ps[:], AF.Copy)
        for cnt in (96, 192):
            cmp_ = pp.tile([64, 128], FP32, name=f"cmps_{cnt}", tag="b6",
                           padded_shape=[128, L])
            nc.tensor.matmul(cmp_[:], s8m[:], bc4[:])
            cm = cp.tile([64, 128], FP32, name=f"CM_{cnt}")
            nc.scalar.activation(r(cm[:]), cmp_[:], AF.Copy, scale=1.0 / cnt)
            CM[cnt] = cm
            cqp = pp.tile([64, 128], FP32, name=f"cqps_{cnt}", tag="b7",
                          padded_shape=[128, L])
            nc.tensor.matmul(cqp[:], s8q[:], bc4[:])
            cq = cp.tile([64, 128], FP32, name=f"CQ_{cnt}")
            nc.scalar.activation(r(cq[:]), cqp[:], AF.Copy, scale=1.0 / cnt)
            CQ[cnt] = cq

    # ================= weights / biases ======================================
    # slot s of the packed wall tile holds W^T zero-padded to [12,16];
    # tp = Wc^T.rsel12 replicates the transpose across groups; sp = tp^T.rsel16
    # spreads along free; masking leaves the block-diagonal lhsT.  u-weights
    # are scaled by a_i here (relu(a z) = a relu(z)).
    WBD = {}
    BIAS = {}
    _wbank = [0]

    def finish_weight(wname, scale=None):
        s = WSLOT[wname]
        tp = pp.tile([GS, 128], FP32, name=f"wt_{wname}", tag="b6",
                     padded_shape=[128, L])
        nc.tensor.matmul(tp[:], wall[:, 16 * s:16 * (s + 1)], rsel12[:])
        ts = sb.tile([GS, 128], FP32, name=f"ws_{wname}", tag="wts")
        nc.scalar.activation(ts[:], tp[:], AF.Copy)
        sp = pp.tile([128, 128], FP32, name=f"wsp_{wname}", tag="b7",
                     padded_shape=[128, L])
        nc.tensor.matmul(sp[:], ts[:], rsel16[:])
        wt = cp.tile([128, 128], FP32, name=f"W_{wname}")
        if scale is None:
            nc.vector.tensor_tensor(r(wt[:]), sp[:], mask_diag[:], OP.mult)
        else:
            nc.vector.scalar_tensor_tensor(
                r(wt[:]), sp[:], scale[:], mask_diag[:], OP.mult, OP.mult)
        WBD[wname] = wt

    def finish_bias(wname, scale=None):
        s = WSLOT[wname]
        bps = pp.tile([128, 1], FP32, name=f"bps_{wname}", tag="b2",
                      padded_shape=[128, L])
        nc.tensor.matmul(bps[:], rsel12[:], ball[:, s:s + 1])
        bt = cp.tile([128, 1], FP32, name=f"bias_{wname}")
        nc.scalar.activation(bt[:], bps[:], AF.Copy)
        if scale is not None:
            bts = cp.tile([128, 1], FP32, name=f"biass_{wname}")
            nc.vector.tensor_tensor(bts[:], bt[:], scale[:], OP.mult)
            bt = bts
        BIAS[wname] = bt[:]

    finish_weight("fc1_w")
    finish_bias("fc1_w")

    bnb_bc, bng_bc = {}, {}

    def build_bn_bc(k):
        bps = pp.tile([128, L], FP32, name=f"bnbps_{k}", tag="b3")
        mmr(bps[:], bc4r[:], bnvec[f"{k}_b"][:])
        bsb = cp.tile([128, L], FP32, name=f"bnbbc_{k}")
        nc.scalar.activation(bsb[:], bps[:], AF.Copy)
        bnb_bc[k] = bsb
        gps = pp.tile([128, L], FP32, name=f"bngps_{k}", tag="b4")
        mmr(gps[:], bc4r[:], bnvec[f"{k}_g"][:])
        gsb = cp.tile([128, L], FP32, name=f"bngbc_{k}")
        nc.scalar.activation(r(gsb[:]), gps[:], AF.Copy)
        bng_bc[k] = gsb

    # ================= helpers ===============================================
    def fc(w, src, name, plain=False):
        ps = pp.tile([128, L], FP32, name=f"psfc_{name}", tag="b0")
        if plain:
            nc.tensor.matmul(ps[:], w[:], src[:])
        else:
            mmr(ps[:], w[:], src[:])
        return ps

    def bn_send(h_ps, bias, tag):
        """fc PSUM -> biased hs + partial stats -> AllGather kickoff."""
        sq = sb.tile([128, L], FP32, name=f"sq_{tag}")
        nc.scalar.activation(r(sq[:]), h_ps[:], AF.Square, bias=bias)
        hs = sb.tile([128, L], FP32, name=f"hs_{tag}")
        nc.vector.tensor_scalar_add(r(hs[:]), h_ps[:], bias)
        pk_q = pp.tile([4, L], FP32, name=f"pkq_{tag}", tag="b2", padded_shape=[128, L])
        pk_s = pp.tile([4, L], FP32, name=f"pks_{tag}", tag="b1", padded_shape=[128, L])
        mmr(pk_q[:], onesfold[:], sq[:])
        mmr(pk_s[:], onesfold[:], hs[:])
        sk_s = sb.tile([4, L], FP32, name=f"sks_{tag}")
        sk_q = sb.tile([4, L], FP32, name=f"skq_{tag}")
        nc.scalar.activation(sk_s[:], pk_s[:], AF.Copy)
        nc.vector.tensor_copy(sk_q[:], pk_q[:])
        cc_in = dr.tile([8, L], FP32, name=f"ccin_{tag}")
        cc_out = dr.tile([64, L], FP32, name=f"ccout_{tag}")
        nc.sync.dma_start(cc_in[4:8, :], sk_q[:])
        nc.scalar.dma_start(cc_in[0:4, :], sk_s[:])
        if single_core:
            # timing-only stand-in for the AllGather (TimelineSim path);
            # 4 serialized DMAs model the ~5us 8-core AllGather latency
            for rr in range(4):
                nc.sync.dma_start(cc_out[8 * rr:8 * rr + 8, :], cc_in[:])
        else:
            nc.gpsimd.collective_compute(
                "AllGather",
                OP.bypass,
                replica_groups=[list(range(NCORES))],
                ins=[cc_in.opt()],
                outs=[cc_out.opt()],
            )
        return hs, cc_out

    def bn_recv(state, key, cnt, tag):
        """Gathered stats -> bn(h) = a*(h-mean)+beta -> relu.

        Stats are reduced AND broadcast to [128,L] in one matmul each via the
        fused CM/CQ matrices; the whole affine chain runs on broadcast tiles.
        """
        hs, cc_out = state
        gath = sb.tile([64, L], FP32, name=f"gath_{tag}")
        nc.sync.dma_start(gath[:].bitcast(F32R), cc_out[:].bitcast(F32R))
        M_bc = pp.tile([128, L], FP32, name=f"Mbc_{tag}", tag="b4")
        mmr(M_bc[:], CM[cnt][:], gath[:])
        Q_bc = pp.tile([128, L], FP32, name=f"Qbc_{tag}", tag="b1")
        mmr(Q_bc[:], CQ[cnt][:], gath[:])
        # msq/var first (critical path); Square on Act - only one PSUM operand
        # is allowed per DVE TensorTensor
        msq = sb.tile([128, L], FP32, name=f"msq_{tag}")
        nc.scalar.activation(msq[:], M_bc[:], AF.Square)
        var = sb.tile([128, L], FP32, name=f"var_{tag}")
        nc.vector.tensor_tensor(var[:], Q_bc[:], msq[:], OP.subtract)
        # h - mean runs during the Act rstd (off the critical path)
        t1 = sb.tile([128, L], FP32, name=f"t1_{tag}")
        nc.vector.tensor_tensor(t1[:], hs[:], M_bc[:], OP.subtract)
        # a = gamma / sqrt(var+eps); Abs_reciprocal_sqrt is the one-op rstd
        # (var+eps > 0 so abs is a no-op)
        rstd = sb.tile([128, L], FP32, name=f"rstd_{tag}")
        nc.scalar.activation(rstd[:], var[:], AF.Abs_reciprocal_sqrt,
                             bias=eps_t[:])
        a = sb.tile([128, L], FP32, name=f"a_{tag}")
        nc.vector.tensor_tensor(a[:], rstd[:], bng_bc[key][:], OP.mult)
        t2 = sb.tile([128, L], FP32, name=f"t2_{tag}")
        nc.vector.tensor_tensor(t2[:], t1[:], a[:], OP.mult)
        t3 = sb.tile([128, L], FP32, name=f"t3_{tag}")
        nc.vector.tensor_tensor(t3[:], t2[:], bnb_bc[key][:], OP.add)
        hn = sb.tile([128, L], FP32, name=f"hn_{tag}")
        nc.vector.tensor_relu(r(hn[:]), t3[:])
        return hn

    def relmod(cur, wu, bu, idx):
        # U' = a*relu(unary(cur)) via the pre-scaled wu/bu
        psU = pp.tile([128, L], FP32, name=f"psU_{idx}", tag="b0")
        mmr(psU[:], wu[:], cur[:])
        # open the xG accumulator early with the +cur identity term so the
        # final output needs only ONE more matmul (Gf) and ONE vector op
        psXG = pp.tile([128, L], FP32, name=f"psXG_{idx}", tag="b7")
        nc.tensor.matmul(psXG[:], identr[:].bitcast(F32R), cur[:].bitcast(F32R), start=True, stop=False)
        U = sb.tile([128, L], FP32, name=f"U_{idx}", tag="U")
        nc.scalar.activation(r(U[:]), psU[:], AF.Relu, bias=bu)
        # s = sum_c cur^2 per token, broadcast to [128,L]
        sq = sb.tile([128, L], FP32, name=f"rsq_{idx}", tag="rsq")
        nc.scalar.activation(r(sq[:]), cur[:], AF.Square)
        psS = pp.tile([NG, L], FP32, name=f"psS_{idx}", tag="b5", padded_shape=[128, L])
        mmr(psS[:], ones_c16[:], sq[:])
        sS = sb.tile([NG, L], FP32, name=f"sS_{idx}", tag="sS")
        nc.vector.tensor_copy(r(sS[:]), psS[:])
        Sbc = pp.tile([128, L], FP32, name=f"Sbc_{idx}", tag="b3")
        mmr(Sbc[:], bc8r[:], sS[:])
        # transposes of cur and U (4x 128-chunks each, f32r for 1.5cyc/row)
        pTc = pp.tile([128, 4 * 128], FP32, name=f"pTc_{idx}", tag="b1")
        pTu = pp.tile([128, 4 * 128], FP32, name=f"pTu_{idx}", tag="b2")
        for j in range(4):
            nc.tensor.transpose(
                pTc[:, 128 * j:128 * (j + 1)].bitcast(F32R),
                cur[:, 128 * j:128 * (j + 1)].bitcast(F32R),
                identr[:].bitcast(F32R))
            nc.tensor.transpose(
                pTu[:, 128 * j:128 * (j + 1)].bitcast(F32R),
                U[:, 128 * j:128 * (j + 1)].bitcast(F32R),
                identr[:].bitcast(F32R))
        curT = sb.tile([128, 4 * 128], FP32, name=f"curT_{idx}", tag="curT")
        nc.scalar.activation(r(curT[:]), pTc[:], AF.Copy)
        if idx == 0:
            # switch the Act tables back to the rsqrt set in relmod1's Act
            # idle window (reading curT pins this after the sigmoid) so bn4's
            # rstd doesn't pay the 1.3us table load on-path
            actwarm2 = sb.tile([1, 1], FP32, name="actwarm2")
            nc.scalar.activation(actwarm2[:], curT[0:1, 0:1],
                                 AF.Abs_reciprocal_sqrt, bias=eps_t[0:1, :])
        # UT copied in halves so psG's accumulation starts one hop earlier
        UTa = sb.tile([128, 256], FP32, name=f"UTa_{idx}", tag="UTa")
        UTb = sb.tile([128, 256], FP32, name=f"UTb_{idx}", tag="UTb")
        nc.vector.tensor_copy(r(UTa[:]), pTu[:, 0:256])
        nc.vector.tensor_copy(r(UTb[:]), pTu[:, 256:512])
        # P' = sum_t U x cur  (per-group partials on diag blocks)
        psG = pp.tile([128, 128], FP32, name=f"psG_{idx}", tag="b4",
                      padded_shape=[128, L])
        for j in range(4):
            ut = UTa if j < 2 else UTb
            mmr(psG[:], ut[:, 128 * (j % 2):128 * (j % 2 + 1)],
                curT[:, 128 * j:128 * (j + 1)],
                start=(j == 0), stop=(j == 3))
        Pm = sb.tile([128, 128], FP32, name=f"Pm_{idx}", tag="Pm")
        nc.vector.tensor_tensor(r(Pm[:]), psG[:], mask_diag[:], OP.mult)
        # G_spread = Phi^T (P_m Phi);  P_m = Pm^T
        psM = pp.tile([128, 128], FP32, name=f"psM_{idx}", tag="b5",
                      padded_shape=[128, L])
        mmr(psM[:], Pm[:], phi[:])
        Ms = sb.tile([128, 128], FP32, name=f"Ms_{idx}", tag="Ms")
        nc.scalar.activation(r(Ms[:]), psM[:], AF.Copy)
        psG2 = pp.tile([128, 128], FP32, name=f"psG2_{idx}", tag="b6",
                       padded_shape=[128, L])
        mmr(psG2[:], phi[:], Ms[:])
        Gf = sb.tile([128, 128], FP32, name=f"Gf_{idx}", tag="Gf")
        nc.vector.tensor_tensor(r(Gf[:]), psG2[:], mask_diag[:], OP.mult)
        # xG + cur lands in the open accumulator
        mmr(psXG[:], Gf[:], cur[:], start=False, stop=True)
        # out = (xG + cur) - s*U   (a already folded into U).  sbc_s/w1 run
        # on Act/Pool so they never steal DVE slots from the UT copies.
        sbc_s = sb.tile([128, L], FP32, name=f"sbcs_{idx}", tag="sbcs")
        nc.scalar.activation(sbc_s[:], Sbc[:], AF.Copy)
        w1 = sb.tile([128, L], FP32, name=f"w1_{idx}", tag="w1")
        nc.gpsimd.tensor_tensor(w1[:], sbc_s[:], U[:], OP.mult)
        nxt = sb.tile([128, L], FP32, name=f"nxt_{idx}", tag="nxt", bufs=2)
        nc.vector.tensor_tensor(r(nxt[:]), psXG[:], w1[:], OP.subtract)
        return nxt

    # ================= network ===============================================
    st1 = bn_send(fc(WBD["fc1_w"], X, "1", plain=True), BIAS["fc1_w"], "bn1")
    # bn1 AllGather window: finish fc2/fc3/u* params, deferred consts
    build_deferred_consts()
    finish_weight("fc2_w")
    finish_bias("fc2_w")
    finish_weight("fc3_w")
    finish_bias("fc3_w")
    for i in range(4):
        finish_weight(f"u{i + 1}_w", scale=a_r[i])
        finish_bias(f"u{i + 1}_w", scale=a_r[i])
    build_bn_bc("bn1")
    build_bn_bc("bn2")
    h1n = bn_recv(st1, "bn1", 96, "bn1")
    st2 = bn_send(fc(WBD["fc2_w"], h1n, "2"), BIAS["fc2_w"], "bn2")
    # bn2 AllGather window: tail params
    finish_weight("fc4_w")
    finish_bias("fc4_w")
    finish_weight("fc5_w")
    finish_bias("fc5_w")
    finish_weight("fc67_w")
    finish_bias("fc67_w")
    build_bn_bc("bn4")
    h2n = bn_recv(st2, "bn2", 192, "bn2")
    ps3 = fc(WBD["fc3_w"], h2n, "3")
    enc_r = sb.tile([128, L], FP32, name="enc_r")
    nc.scalar.activation(enc_r[:], ps3[:], AF.Sigmoid, bias=BIAS["fc3_w"])
    # zero the c>=12 garbage rows (sigmoid(0)=0.5) so downstream sums are clean
    enc = sb.tile([128, L], FP32, name="enc")
    nc.vector.tensor_scalar_mul(r(enc[:]), enc_r[:], colmask12[:])

    cur = enc
    for i in range(4):
        cur = relmod(cur, WBD[f"u{i + 1}_w"], BIAS[f"u{i + 1}_w"], i)

    # b67[(b,c'),0] = fc67 bias per output channel (b-independent)
    selb = cp.tile([128, 6], FP32, name="selb")
    affine_sel(selb[:].rearrange("p (b c) -> p b c", c=3), [[0, 2], [1, 3]], -1)
    b67ps = pp.tile([6, 1], FP32, name="b67ps", tag="b2", padded_shape=[128, L])
    nc.tensor.matmul(b67ps[:], selb[:], BIAS["fc67_w"])
    b67 = cp.tile([6, 1], FP32, name="b67")
    nc.scalar.activation(b67[:], b67ps[:], AF.Copy)

    st4 = bn_send(fc(WBD["fc4_w"], cur, "4"), BIAS["fc4_w"], "bn4")
    h4n = bn_recv(st4, "bn4", 96, "bn4")
    ps5 = fc(WBD["fc5_w"], h4n, "5")
    h5 = sb.tile([128, L], FP32, name="h5")
    nc.scalar.activation(r(h5[:]), ps5[:], AF.Relu, bias=BIAS["fc5_w"])
    # fused fc67: per quarter q, contract with the column slice of W_fc67
    # whose outputs are rows {64b+16q+c'} - the result lands directly in the
    # DRAM [3,2048]-per-batch layout, so the store is 2 contiguous DMAs
    oraw = sb.tile([6, 4 * L], FP32, name="oraw")
    w67 = WBD["fc67_w"][:].rearrange("p (b rest) -> p b rest", b=2)
    w67q = []
    for q in range(4):
        t = cp.tile([128, 6], FP32, name=f"w67q_{q}")
        nc.vector.tensor_copy(r(t[:]), w67[:, :, 16 * q:16 * q + F])
        w67q.append(t)
    for q in range(4):
        psq = pp.tile([6, L], FP32, name=f"psraw_{q}", tag=f"b{4 + q}",
                      padded_shape=[128, L])
        mmr(psq[:], w67q[q][:], h5[:])
        if q % 2 == 0:
            nc.scalar.add(oraw[:, L * q:L * (q + 1)], psq[:], b67[:])
        else:
            nc.vector.tensor_scalar_add(oraw[:, L * q:L * (q + 1)], psq[:], b67[:])
    for b in range(BPC):
        eng = nc.sync if b % 2 == 0 else nc.scalar
        eng.dma_start(
            out_d[b][:, :],
            oraw[F * b:F * b + F, :].rearrange("c (q f) -> c (q f)", q=4))


_PROGRAM = None


def _get_program():
    global _PROGRAM
    if _PROGRAM is None:
        _PROGRAM = _build()
    return _PROGRAM


def _pack_params(inputs):
    """Host-side LAYOUT-ONLY packing of the tiny weights (no arithmetic)."""
    wall = np.zeros((D4, 16 * len(W_ORDER)), np.float32)
    ball = np.zeros((D4, len(W_ORDER)), np.float32)
    for s, name in enumerate(W_ORDER):
        slot = np.zeros((D4, 16), np.float32)
        if name == "fc67_w":
            slot[0:F, 0:1] = inputs["fc6_w"].T
            slot[0:F, 1:3] = inputs["fc7_w"].T
            ball[0:1, s] = inputs["fc6_b"]
            ball[1:3, s] = inputs["fc7_b"]
        else:
            w = inputs[name]
            o, i = w.shape
            slot[0:i, 0:o] = w.T
            ball[0:o, s] = inputs[name.replace("_w", "_b")]
        wall[:, 16 * s:16 * (s + 1)] = slot
    relsc = np.concatenate([
        np.concatenate([inputs[f"ps{i}"] for i in range(1, 5)]),
        np.concatenate([inputs[f"ph{i}"] for i in range(1, 5)]),
        np.concatenate([inputs[f"wr{i}"] for i in range(1, 5)]),
    ]).astype(np.float32)
    return (np.ascontiguousarray(wall), np.ascontiguousarray(ball),
            np.ascontiguousarray(relsc))


def run(inputs, trace=False, **kw):
    inputs = {k: np.asarray(v, np.float32) for k, v in inputs.items()}
    nc = _get_program()
    wall, ball, relsc = _pack_params(inputs)
    base = {"wall": wall, "ball": ball, "relsc": relsc}
    for name in BN_VECS:
        base[name] = np.ascontiguousarray(inputs[name])
    in_maps = []
    for i in range(NCORES):
        m = dict(base)
        # feed x as [BPC, F, N] so the load DMA is contiguous (layout only)
        m["x"] = np.ascontiguousarray(
            inputs["x"][BPC * i:BPC * (i + 1)].transpose(0, 2, 1))
        in_maps.append(m)
    last_exc = None
    for attempt in range(3):
        try:
            res = run_bass_kernel_spmd(
                nc, in_maps, core_ids=list(range(NCORES)), trace=trace, **kw)
            break
        except Exception as e:  # transient NRT_EXEC_UNIT_UNRECOVERABLE flakes
            last_exc = e
            import time
            time.sleep(5)
    else:
        raise last_exc
    out = np.concatenate(
        [res.results[i]["out"].transpose(0, 2, 1) for i in range(NCORES)],
        axis=0)
    return np.ascontiguousarray(out), res


def kernel(**inputs) -> np.ndarray:
    out, _ = run(inputs)
    return out
